# revision 1
# baseline (speedup 1.0000x reference)
"""Trainium2 Bass kernel for nn_ExpWindowAttention (windowed sparse attention).

Strategy: pure data-parallel over batch (32 -> 8 cores x 4 batches).

Key algebraic fusion: only probs = (o @ wo.T + bo) @ wh.T + bh is consumed
(nC=2), so the whole post-softmax pipeline collapses to rank-2 per head:
  probs[w, c] = sum_{h,t} attn[w, h, t] * vp[5w + t, h, c] + BC[c]
  vp = x @ wvp.T,  wvp[(h, c), :] = (wh @ wo)[c, hslice] @ wv[hslice, :]
  BC = wh @ (wo @ bv + bo) + bh
This eliminates the V projection, attention-times-V, out-projection, and
head GEMMs entirely; vp is a 16-wide projection and the banded contraction
runs on the vector engine.

Q/K projections run in fp8 (e4m3) with DoubleRow matmuls (2 contraction
rows/cycle). Weights are pre-scaled by 128 into e4m3's normal range; the
resulting 128^2 score scale folds into the softmax exp scale. vp uses
bf16 weights against the fp8 x for accuracy (it feeds probs directly).

Per (batch, window-chunk): project K from a position segment, compute
dense per-head scores, extract the 11-wide diagonal band with a flat-AP
DMA gather, softmax in the compact band domain, band-gather vp and
contract attn x vp on the vector engine, then log_softmax + scatter.
"""

import numpy as np
import ml_dtypes

import concourse.bass as bass
from concourse import bacc
import concourse.mybir as mybir
import concourse.tile as tile
from concourse.bass_utils import run_bass_kernel_spmd

F32 = mybir.dt.float32
BF16 = mybir.dt.bfloat16
F8 = mybir.dt.float8e4
DR = mybir.MatmulPerfMode.DoubleRow
WS8 = 128.0              # fp8 weight pre-scale for q/k projections

NCORES = 8
B = 32
BL = B // NCORES          # batches per core
L = 2048
D = 1024
H = 8
HD = 128
W = 5
T = 2 * W + 1             # 11
NW = 409                  # windows per batch
NWQ = 416                 # padded center count (32-mult)
NC = 2
LP = 2176                 # padded position count (17*128)
SCALE = float(1.0 / np.sqrt(HD))
SSCALE = float(SCALE / (WS8 * WS8))   # scores carry WS8^2 from fp8 q/k scaling
NEGLOG2 = float(-np.log(2.0))

# (window_start, window_count, seg_start, seg_len)
CHUNKS = []
for c in range(5):
    if c < 4:
        CHUNKS.append((96 * c, 96, 480 * c, 512))
    else:
        CHUNKS.append((96 * c, 25, 1920, 256))
SLK = [512, 512, 512, 512, 136]  # K/scores width per chunk (tail trimmed)


def _flat_ap(t, extra_offset, dims):
    """AP over a tile's backing tensor flat element space (partition-major)."""
    return bass.AP(tensor=t.tensor, offset=t.offset + extra_offset,
                   ap=[list(d) for d in dims])


def _rowsz(t):
    """True per-partition stride (elements) of a tile, from its own AP."""
    return int(t[:].ap[0][0])


def _build(debug=False):
    nc = bacc.Bacc(None, target_bir_lowering=False)

    xt_d = nc.declare_dram_parameter("xt", [BL, D, LP], F8, isOutput=False)
    xq_d = nc.declare_dram_parameter("xq", [BL, D, NWQ], F8, isOutput=False)
    wqk_d = nc.declare_dram_parameter("wqkt", [D, 2 * D], F8, isOutput=False)
    wvp_d = nc.declare_dram_parameter("wvpt", [D, 2 * H], BF16, isOutput=False)
    bqk_d = nc.declare_dram_parameter("bqk", [2 * D], F32, isOutput=False)
    bc2_d = nc.declare_dram_parameter("bc2", [NC], F32, isOutput=False)
    out_d = nc.declare_dram_parameter("out", [BL, L, NC], F32, isOutput=True)
    if debug:
        dbg_vps = nc.declare_dram_parameter("dbg_vps", [2 * H, 512], F32, isOutput=True)
        dbg_vpb = nc.declare_dram_parameter("dbg_vpb", [96, NC, 8, T], F32, isOutput=True)
        dbg_attn = nc.declare_dram_parameter("dbg_attn", [96, 8, T], F32, isOutput=True)
        dbg_pb = nc.declare_dram_parameter("dbg_pb", [96, 5, NC], F32, isOutput=True)
        dbg_kt = nc.declare_dram_parameter("dbg_kt", [128, 8, 512], BF16, isOutput=True)

    with tile.TileContext(nc) as tc:
        import contextlib
        with contextlib.ExitStack() as ctx:
            const = ctx.enter_context(tc.tile_pool(name="const", bufs=1))
            perb = ctx.enter_context(tc.tile_pool(name="perb", bufs=2))
            perb1 = ctx.enter_context(tc.tile_pool(name="perb1", bufs=1))
            perb2 = ctx.enter_context(tc.tile_pool(name="perb2", bufs=2))
            xtp = ctx.enter_context(tc.tile_pool(name="xtp", bufs=3))
            ktp = ctx.enter_context(tc.tile_pool(name="ktp", bufs=2))
            vpsp = ctx.enter_context(tc.tile_pool(name="vpsp", bufs=2))
            ssbp = ctx.enter_context(tc.tile_pool(name="ssbp", bufs=3))
            drp = ctx.enter_context(tc.tile_pool(name="drp", bufs=2, space="DRAM"))
            smx = ctx.enter_context(tc.tile_pool(name="smx", bufs=2))
            proj_ps = ctx.enter_context(tc.tile_pool(name="proj_ps", bufs=3, space="PSUM"))
            vp_ps = ctx.enter_context(tc.tile_pool(name="vp_ps", bufs=2, space="PSUM"))
            sc_ps = ctx.enter_context(tc.tile_pool(name="sc_ps", bufs=3, space="PSUM"))

            # ---- resident weights / biases ----
            wqk = const.tile([128, 8, 2 * D], F8)
            nc.sync.dma_start(
                out=wqk, in_=wqk_d.rearrange("(kc p) c -> p kc c", p=128)
            )
            wvp = const.tile([128, 8, 2 * H], BF16)
            nc.sync.dma_start(
                out=wvp, in_=wvp_d.rearrange("(kc p) c -> p kc c", p=128)
            )
            bqk_col = const.tile([128, 16], F32)  # [p, proj*8+fc]
            nc.sync.dma_start(out=bqk_col, in_=bqk_d.rearrange("(c p) -> p c", p=128))
            bias2 = const.tile([128, NC], F32)    # BC broadcast along partitions
            nc.sync.dma_start(
                out=bias2, in_=bass.AP(tensor=bc2_d, offset=0, ap=[[0, 128], [1, NC]])
            )
            cfill = const.tile([128, 8], F32)
            nc.vector.memset(cfill, NEGLOG2)

            state = {}

            def qproj(b):
                """Per-batch Q projection from host-pregathered centers."""
                xq = perb1.tile([128, 8, NWQ], F8, tag="xq")
                nc.sync.dma_start(
                    out=xq, in_=xq_d[b].rearrange("(kc p) c -> p kc c", p=128)
                )
                qt = perb.tile([128, 8, NWQ], BF16, tag="qt")
                for h in range(8):
                    qps = proj_ps.tile([128, NWQ], F32, tag="pps")
                    for j in range(4):
                        nc.tensor.matmul(
                            qps[:],
                            wqk[:, 2 * j : 2 * j + 2, h * 128 : h * 128 + 128],
                            xq[:, 2 * j : 2 * j + 2, :],
                            start=(j == 0),
                            stop=(j == 3),
                            perf_mode=DR,
                        )
                    if h % 2 == 0:
                        nc.vector.tensor_scalar_add(
                            qt[:, h, :], qps[:], bqk_col[:, h : h + 1]
                        )
                    else:
                        nc.scalar.add(qt[:, h, :], qps[:], bqk_col[:, h : h + 1])
                state[("qt", b)] = qt

            def produce(b, ci):
                """K projection + vp (rank-2 value path) for one (batch, chunk)."""
                ws, wcnt, ss, sl = CHUNKS[ci]
                xt = xtp.tile([128, 8, sl], F8, tag="xt")
                nc.sync.dma_start(
                    out=xt,
                    in_=xt_d[b].rearrange("(kc p) c -> p kc c", p=128)[
                        :, :, ss : ss + sl
                    ],
                )
                slk = SLK[ci]
                kt = ktp.tile([128, 8, slk], BF16, tag="kt")
                for h in range(8):
                    kps = proj_ps.tile([128, slk], F32, tag="pps")
                    for j in range(4):
                        nc.tensor.matmul(
                            kps[:],
                            wqk[:, 2 * j : 2 * j + 2, D + h * 128 : D + h * 128 + 128],
                            xt[:, 2 * j : 2 * j + 2, 0:slk],
                            start=(j == 0),
                            stop=(j == 3),
                            perf_mode=DR,
                        )
                    if h < 3:
                        nc.vector.tensor_scalar_add(
                            kt[:, h, :], kps[:], bqk_col[:, 8 + h : 9 + h]
                        )
                    else:
                        nc.scalar.add(kt[:, h, :], kps[:], bqk_col[:, 8 + h : 9 + h])
                # vp chunk: [16, sl] = wvp.T @ x  (bf16 weights x fp8 x),
                # staged to DRAM as [c, pos, h] so the band read is contiguous
                vp_d = state[("vp_d", b)]
                vps = vp_ps.tile([2 * H, sl], F32, tag="vps")
                for kc in range(8):
                    nc.tensor.matmul(
                        vps[:],
                        wvp[:, kc, :],
                        xt[:, kc, :],
                        start=(kc == 0),
                        stop=(kc == 7),
                    )
                vpsb = vpsp.tile([2 * H, sl], F32, tag="vpsb")
                nc.vector.tensor_copy(vpsb[:], vps[:])
                rsv = _rowsz(vpsb)
                for c in range(NC):
                    nc.sync.dma_start(
                        out=_flat_ap(
                            vp_d, c * 8 * LP + ss, [[LP, 8], [1, sl]]
                        ),
                        in_=_flat_ap(vpsb, c * rsv, [[2 * rsv, 8], [1, sl]]),
                    )
                if debug and b == 0 and ci == 0:
                    nc.sync.dma_start(out=dbg_vps[:, :], in_=vpsb[:])
                    nc.sync.dma_start(out=dbg_kt[:, :, :], in_=kt[:])
                state[("kt", b, ci)] = kt

            def attend(b, ci):
                ws, wcnt, ss, sl = CHUNKS[ci]
                qt = state[("qt", b)]
                kt = state.pop(("kt", b, ci))

                # scores per head + band gather
                slk = SLK[ci]
                band = smx.tile([wcnt, 8, T], F32, tag="band")
                for h in range(8):
                    sps = sc_ps.tile([wcnt, slk], F32, tag="sps")
                    nc.tensor.matmul(
                        sps[:], qt[:, h, ws : ws + wcnt], kt[:, h, :],
                        start=True, stop=True,
                    )
                    ssb = ssbp.tile([wcnt, slk], F32, tag="ssb")
                    if h < 5:
                        nc.vector.tensor_copy(ssb[:], sps[:])
                    else:
                        nc.scalar.copy(ssb[:], sps[:])
                    rs = _rowsz(ssb)
                    nc.sync.dma_start(
                        out=band[:, h, :],
                        in_=_flat_ap(ssb, 0, [[rs + 5, wcnt], [1, T]]),
                    )

                # softmax in band domain: subtract per-(w,h) max on gpsimd,
                # one fused Exp on scalar engine, sums via gpsimd reduce
                negmax = smx.tile([wcnt, 8], F32, tag="negmax")
                nc.vector.tensor_reduce(
                    negmax[:], band[:], axis=mybir.AxisListType.X,
                    op=mybir.AluOpType.max, negate=True,
                )
                ebs = smx.tile([wcnt, 8, T], F32, tag="ebs")
                negmax_bc = negmax[:].unsqueeze(2).to_broadcast([wcnt, 8, T])
                nc.gpsimd.tensor_tensor(
                    out=ebs[:], in0=band[:], in1=negmax_bc, op=mybir.AluOpType.add
                )
                eb = smx.tile([wcnt, 8, T], F32, tag="eb")
                nc.scalar.activation(
                    eb[:], ebs[:], mybir.ActivationFunctionType.Exp,
                    bias=0.0, scale=SSCALE,
                )
                sums = smx.tile([wcnt, 8], F32, tag="sums")
                nc.vector.tensor_reduce(
                    sums[:], eb[:], axis=mybir.AxisListType.X,
                    op=mybir.AluOpType.add,
                )
                recip = smx.tile([wcnt, 8], F32, tag="recip")
                nc.vector.reciprocal(recip[:], sums[:])
                attn = smx.tile([wcnt, 8, T], F32, tag="attn")
                recip_bc = recip[:].unsqueeze(2).to_broadcast([wcnt, 8, T])
                nc.gpsimd.tensor_tensor(
                    out=attn[:], in0=eb[:], in1=recip_bc, op=mybir.AluOpType.mult
                )

                # banded value contraction: probs[w, c] = sum_{h,t} attn * vp
                vp_d = state[("vp_d", b)]
                vpb = smx.tile([wcnt, NC, 8, T], F32, tag="vpb")
                for c in range(NC):
                    nc.sync.dma_start(
                        out=vpb[:, c, :, :],
                        in_=_flat_ap(
                            vp_d, 480 * ci + c * 8 * LP,
                            [[5, wcnt], [LP, 8], [1, T]],
                        ),
                    )
                prod = smx.tile([wcnt, NC, 8, T], F32, tag="prod")
                attn_bc = attn[:].unsqueeze(1).to_broadcast([wcnt, NC, 8, T])
                nc.gpsimd.tensor_tensor(
                    out=prod[:], in0=vpb[:], in1=attn_bc, op=mybir.AluOpType.mult
                )
                red1 = smx.tile([wcnt, NC, 8], F32, tag="red1")
                nc.vector.tensor_reduce(
                    red1[:], prod[:], axis=mybir.AxisListType.X,
                    op=mybir.AluOpType.add,
                )
                pb = state[("pb", b)]
                nc.vector.tensor_reduce(
                    pb[0:wcnt, ci, :], red1[:], axis=mybir.AxisListType.X,
                    op=mybir.AluOpType.add,
                )
                if debug and b == 0 and ci == 0:
                    nc.sync.dma_start(out=dbg_vpb[:, :, :, :], in_=vpb[:])
                    nc.sync.dma_start(out=dbg_attn[:, :, :], in_=attn[:])

            def tail(b):
                """bias + log_softmax + output DMA for one batch."""
                pb = state.pop(("pb", b))
                state.pop(("qt", b))
                state.pop(("vp_d", b))
                if debug and b == 0:
                    nc.sync.dma_start(out=dbg_pb[:, :, :], in_=pb[:])
                pb2 = perb2.tile([96, 5, NC], F32, tag="pb2")
                bias_bc = bias2[0:96, :].unsqueeze(1).to_broadcast([96, 5, NC])
                nc.gpsimd.tensor_tensor(
                    out=pb2[:], in0=pb[:], in1=bias_bc, op=mybir.AluOpType.add
                )

                pmax = perb2.tile([96, 5], F32, tag="pmax")
                nc.vector.tensor_reduce(
                    pmax[:], pb2[:], axis=mybir.AxisListType.X, op=mybir.AluOpType.max
                )
                nmax = perb2.tile([96, 5], F32, tag="nmax")
                nc.gpsimd.tensor_scalar_mul(nmax[:], pmax[:], -1.0)
                sexp = perb2.tile([96, 5], F32, tag="sexp")
                etmp = perb2.tile([96, 5, NC], F32, tag="etmp")
                for wc in range(5):
                    nc.scalar.activation(
                        etmp[:, wc, :], pb2[:, wc, :],
                        mybir.ActivationFunctionType.Exp,
                        bias=nmax[:, wc : wc + 1], scale=1.0,
                        accum_out=sexp[:, wc : wc + 1],
                    )
                lns = perb2.tile([96, 5], F32, tag="lns")
                nc.scalar.activation(lns[:], sexp[:], mybir.ActivationFunctionType.Ln)
                otm = perb2.tile([96, 5, NC], F32, tag="otm")
                for wc in range(5):
                    nc.gpsimd.tensor_scalar(
                        out=otm[:, wc, :],
                        in0=pb2[:, wc, :],
                        scalar1=pmax[:, wc : wc + 1],
                        scalar2=lns[:, wc : wc + 1],
                        op0=mybir.AluOpType.subtract,
                        op1=mybir.AluOpType.subtract,
                    )

                # write output: const fill + strided scatter
                nc.sync.dma_start(
                    out=bass.AP(
                        tensor=out_d, offset=b * L * NC + 2,
                        ap=[[1, 1], [10, NW], [1, 8]],
                    ),
                    in_=_flat_ap(cfill, 0, [[_rowsz(cfill), 1], [0, NW], [1, 8]]),
                )
                nc.sync.dma_start(
                    out=bass.AP(
                        tensor=out_d, offset=b * L * NC + 2045 * NC, ap=[[1, 6]]
                    ),
                    in_=cfill[0:1, 0:6],
                )
                rt = _rowsz(otm)
                nc.sync.dma_start(
                    out=bass.AP(
                        tensor=out_d, offset=b * L * NC,
                        ap=[[10, 96], [960, 4], [1, NC]],
                    ),
                    in_=_flat_ap(otm, 0, [[rt, 96], [NC, 4], [1, NC]]),
                )
                nc.sync.dma_start(
                    out=bass.AP(
                        tensor=out_d, offset=b * L * NC + 3840, ap=[[10, 25], [1, NC]]
                    ),
                    in_=_flat_ap(otm, 4 * NC, [[rt, 25], [1, NC]]),
                )

            def start_batch(b):
                qproj(b)
                state[("vp_d", b)] = drp.tile(
                    [NC, 8, LP], F32, tag="vp_d", name="vp_d"
                )
                state[("pb", b)] = perb.tile([96, 5, NC], F32, tag="pb", name="pb")

            # ---- software-pipelined emission (2-deep) ----
            iters = [(b, ci) for b in range(BL) for ci in range(5)]
            start_batch(0)
            produce(0, 0)
            for i in range(1, len(iters)):
                b, ci = iters[i]
                if ci == 0:
                    start_batch(b)
                produce(b, ci)
                pb_, pci = iters[i - 1]
                attend(pb_, pci)
                if pci == 4:
                    tail(pb_)
            lb, lci = iters[-1]
            attend(lb, lci)
            tail(lb)

    nc.compile()
    return nc


_NC_CACHE = {}


def _get_nc(debug=False):
    if debug not in _NC_CACHE:
        _NC_CACHE[debug] = _build(debug)
    return _NC_CACHE[debug]


def kernel(x, in_proj_w, in_proj_b, out_proj_w, out_proj_b, out_w, out_b, x_len=None,
           _want_perf=False, _debug=False):
    x = np.asarray(x, dtype=np.float32)
    in_proj_w = np.asarray(in_proj_w, dtype=np.float32)
    in_proj_b = np.asarray(in_proj_b, dtype=np.float32)
    out_proj_w = np.asarray(out_proj_w, dtype=np.float32)
    out_proj_b = np.asarray(out_proj_b, dtype=np.float32)
    out_w = np.asarray(out_w, dtype=np.float32)
    out_b = np.asarray(out_b, dtype=np.float32)

    nc = _get_nc(_debug)

    # host-side layout prep; x and q/k weights go to fp8 (e4m3) for
    # DoubleRow matmuls, with weights pre-scaled by WS8 to center the
    # small-magnitude weight distribution in e4m3's normal range.
    xt = np.zeros((B, D, LP), dtype=ml_dtypes.float8_e4m3)
    xt[:, :, :L] = x.transpose(0, 2, 1).astype(ml_dtypes.float8_e4m3)
    xq = np.zeros((B, D, NWQ), dtype=ml_dtypes.float8_e4m3)
    xq[:, :, :NW] = xt[:, :, 5 : 5 * NW + 5 : 5]
    wqkt = np.ascontiguousarray(
        in_proj_w[: 2 * D].T * np.float32(WS8)
    ).astype(ml_dtypes.float8_e4m3)

    # fused rank-2 value path (computed in float64 host-side):
    # wvp[(h,c), :] = (wh @ wo)[c, hslice] @ wv[hslice, :]
    wv = in_proj_w[2 * D :].astype(np.float64)
    bv = in_proj_b[2 * D :].astype(np.float64)
    wo64 = out_proj_w.astype(np.float64)
    wh64 = out_w.astype(np.float64)
    wf = wh64 @ wo64                      # [2, D]
    wvp = np.zeros((2 * H, D), np.float64)
    for h in range(H):
        sl_ = slice(h * HD, (h + 1) * HD)
        wvp[2 * h : 2 * h + 2] = wf[:, sl_] @ wv[sl_, :]
    bc2 = (wh64 @ (wo64 @ bv + out_proj_b.astype(np.float64))
           + out_b.astype(np.float64)).astype(np.float32)
    wvpt = np.ascontiguousarray(wvp.T).astype(ml_dtypes.bfloat16)

    in_maps = []
    for c in range(NCORES):
        in_maps.append({
            "xt": np.ascontiguousarray(xt[c * BL : (c + 1) * BL]),
            "xq": np.ascontiguousarray(xq[c * BL : (c + 1) * BL]),
            "wqkt": wqkt,
            "wvpt": wvpt,
            "bqk": in_proj_b[: 2 * D] * np.float32(WS8),
            "bc2": bc2,
        })

    if _debug:
        kr = run_bass_kernel_spmd(nc, in_maps[:1], core_ids=[0])
        return kr.results[0]
    kr = run_bass_kernel_spmd(
        nc, in_maps, core_ids=list(range(NCORES)), trace=_want_perf
    )
    out = np.concatenate([r["out"] for r in kr.results], axis=0).reshape(-1, NC)
    if _want_perf:
        return out, kr
    return out



# revision 12
# speedup vs baseline: 1.2473x; 1.2473x over previous
"""Trainium2 Bass kernel for nn_ExpWindowAttention (windowed sparse attention).

Strategy: pure data-parallel over batch (32 -> 8 cores x 4 batches).

Key algebraic fusion: only probs = (o @ wo.T + bo) @ wh.T + bh is consumed
(nC=2), so the whole post-softmax pipeline collapses to rank-2 per head:
  probs[w, c] = sum_{h,t} attn[w, h, t] * vp[5w + t, h, c] + BC[c]
  vp = x @ wvp.T,  wvp[(h, c), :] = (wh @ wo)[c, hslice] @ wv[hslice, :]
This eliminates the V projection, attention-times-V, out-projection, and
head GEMMs entirely; vp is a 16-wide projection and the banded contraction
runs on the vector engine.

Q/K/vp projections run in fp8 (e4m3) with DoubleRow matmuls (2 contraction
rows/cycle). Weights are pre-scaled into e4m3's normal range; the score
scale folds into the softmax exp scale and the vp scale into the PSUM
evacuation copy / softmax reciprocal.

Per (batch, window-chunk): project K from a position segment, compute
dense per-head scores, extract the 11-wide diagonal band with a single
flat-AP DMA gather over all heads, softmax in the compact band domain,
band-gather vp (SBUF-resident) and contract attn x vp on the vector
engine. The final log_softmax over nC=2 uses Softplus (deferred to the
end so the scalar engine's Exp activation table is never swapped out
mid-softmax).
"""

import numpy as np
import ml_dtypes

import concourse.bass as bass
from concourse import bacc
import concourse.mybir as mybir
import concourse.tile as tile
from concourse.bass_utils import run_bass_kernel_spmd

F32 = mybir.dt.float32
BF16 = mybir.dt.bfloat16
F8 = mybir.dt.float8e4
DR = mybir.MatmulPerfMode.DoubleRow
WS8 = 128.0              # fp8 weight pre-scale for q/k projections
VS8 = 1024.0             # fp8 weight pre-scale for the vp projection

NCORES = 8
B = 32
BL = B // NCORES          # batches per core
L = 2048
D = 1024
H = 8
HD = 128
W = 5
T = 2 * W + 1             # 11
NW = 409                  # windows per batch
NWQ = 416                 # padded center count (32-mult)
NC = 2
LP = 2176                 # padded position count (17*128)
SCALE = float(1.0 / np.sqrt(HD))
SSCALE = float(SCALE / (WS8 * WS8))   # scores carry WS8^2 from fp8 q/k scaling
NEGLOG2 = float(-np.log(2.0))

# (window_start, window_count, seg_start, seg_len)
CHUNKS = []
for c in range(5):
    if c < 4:
        CHUNKS.append((96 * c, 96, 480 * c, 512))
    else:
        CHUNKS.append((96 * c, 25, 1920, 256))
SLK = [512, 512, 512, 512, 136]  # K/scores width per chunk (tail trimmed)


def _flat_ap(t, extra_offset, dims):
    """AP over a tile's backing tensor flat element space (partition-major)."""
    return bass.AP(tensor=t.tensor, offset=t.offset + extra_offset,
                   ap=[list(d) for d in dims])


def _rowsz(t):
    """True per-partition stride (elements) of a tile, from its own AP."""
    return int(t[:].ap[0][0])


def _build(debug=False):
    nc = bacc.Bacc(None, target_bir_lowering=False)

    xt_d = nc.declare_dram_parameter("xt", [BL, D, LP], F8, isOutput=False)
    xq_d = nc.declare_dram_parameter("xq", [BL, D, NWQ], F8, isOutput=False)
    wqk_d = nc.declare_dram_parameter("wqkt", [D, 2 * D], F8, isOutput=False)
    wvp_d = nc.declare_dram_parameter("wvpt", [D, 2 * H], F8, isOutput=False)
    bqk_d = nc.declare_dram_parameter("bqk", [2 * D], F32, isOutput=False)
    bc2_d = nc.declare_dram_parameter("bc2", [NC], F32, isOutput=False)
    out_d = nc.declare_dram_parameter("out", [BL, L, NC], F32, isOutput=True)

    with tile.TileContext(nc) as tc:
        import contextlib
        with contextlib.ExitStack() as ctx:
            const = ctx.enter_context(tc.tile_pool(name="const", bufs=1))
            perb = ctx.enter_context(tc.tile_pool(name="perb", bufs=2))
            perb2 = ctx.enter_context(tc.tile_pool(name="perb2", bufs=2))
            tailp = ctx.enter_context(tc.tile_pool(name="tailp", bufs=4))
            xtp = ctx.enter_context(tc.tile_pool(name="xtp", bufs=2))
            ktp = ctx.enter_context(tc.tile_pool(name="ktp", bufs=2))
            vpsp = ctx.enter_context(tc.tile_pool(name="vpsp", bufs=2))
            drp = ctx.enter_context(tc.tile_pool(name="drp", bufs=2, space="DRAM"))
            ssbp = ctx.enter_context(tc.tile_pool(name="ssbp", bufs=2))
            smx = ctx.enter_context(tc.tile_pool(name="smx", bufs=2))
            proj_ps = ctx.enter_context(tc.tile_pool(name="proj_ps", bufs=3, space="PSUM"))
            vp_ps = ctx.enter_context(tc.tile_pool(name="vp_ps", bufs=2, space="PSUM"))
            sc_ps = ctx.enter_context(tc.tile_pool(name="sc_ps", bufs=3, space="PSUM"))

            # ---- resident weights / biases ----
            wqk = const.tile([128, 8, 2 * D], F8)
            nc.sync.dma_start(
                out=wqk, in_=wqk_d.rearrange("(kc p) c -> p kc c", p=128)
            )
            wvp = const.tile([128, 8, 2 * H], F8)
            nc.sync.dma_start(
                out=wvp, in_=wvp_d.rearrange("(kc p) c -> p kc c", p=128)
            )
            bqk_col = const.tile([128, 16], F32)  # [p, proj*8+fc]
            nc.sync.dma_start(out=bqk_col, in_=bqk_d.rearrange("(c p) -> p c", p=128))
            bias2 = const.tile([128, NC], F32)    # BC broadcast along partitions
            nc.sync.dma_start(
                out=bias2, in_=bass.AP(tensor=bc2_d, offset=0, ap=[[0, 128], [1, NC]])
            )
            cfill = const.tile([128, 8], F32)
            nc.vector.memset(cfill, NEGLOG2)

            # non-window output rows: constant log(1/2) fill, independent of
            # all compute -> issue immediately for full overlap
            for b in range(BL):
                nc.sync.dma_start(
                    out=bass.AP(
                        tensor=out_d, offset=b * L * NC + 2,
                        ap=[[1, 1], [10, NW], [1, 8]],
                    ),
                    in_=_flat_ap(cfill, 0, [[_rowsz(cfill), 1], [0, NW], [1, 8]]),
                )
                nc.sync.dma_start(
                    out=bass.AP(
                        tensor=out_d, offset=b * L * NC + 2045 * NC, ap=[[1, 6]]
                    ),
                    in_=cfill[0:1, 0:6],
                )

            # all batches' pre-gathered query centers in one resident tile
            xqall = const.tile([128, 8, BL * NWQ], F8)
            for b in range(BL):
                nc.sync.dma_start(
                    out=xqall[:, :, b * NWQ : (b + 1) * NWQ],
                    in_=xq_d[b].rearrange("(kc p) c -> p kc c", p=128),
                )

            state = {}

            def qproj(b):
                """Per-batch Q projection from host-pregathered centers."""
                qt = perb.tile([128, 8, NWQ], BF16, tag="qt")
                for h in range(8):
                    qps = proj_ps.tile([128, NWQ], F32, tag="pps")
                    for j in range(4):
                        nc.tensor.matmul(
                            qps[:],
                            wqk[:, 2 * j : 2 * j + 2, h * 128 : h * 128 + 128],
                            xqall[:, 2 * j : 2 * j + 2, b * NWQ : b * NWQ + NWQ],
                            start=(j == 0),
                            stop=(j == 3),
                            perf_mode=DR,
                        )
                    if h % 2 == 0:
                        nc.vector.tensor_scalar_add(
                            qt[:, h, :], qps[:], bqk_col[:, h : h + 1]
                        )
                    else:
                        nc.scalar.add(qt[:, h, :], qps[:], bqk_col[:, h : h + 1])
                state[("qt", b)] = qt

            def load_x(b):
                """Whole-batch transposed-x load (one large DMA)."""
                xt = xtp.tile([128, 8, LP], F8, tag="xt")
                nc.sync.dma_start(
                    out=xt, in_=xt_d[b].rearrange("(kc p) c -> p kc c", p=128)
                )
                state[("xt", b)] = xt

            def produce(b, ci):
                """K projection + vp (rank-2 value path) for one (batch, chunk)."""
                ws, wcnt, ss, sl = CHUNKS[ci]
                xt = state[("xt", b)]
                slk = SLK[ci]
                kt = ktp.tile([128, 8, slk], BF16, tag="kt")
                for h in range(8):
                    kps = proj_ps.tile([128, slk], F32, tag="pps")
                    for j in range(4):
                        nc.tensor.matmul(
                            kps[:],
                            wqk[:, 2 * j : 2 * j + 2, D + h * 128 : D + h * 128 + 128],
                            xt[:, 2 * j : 2 * j + 2, ss : ss + slk],
                            start=(j == 0),
                            stop=(j == 3),
                            perf_mode=DR,
                        )
                    if h < 3:
                        nc.vector.tensor_scalar_add(
                            kt[:, h, :], kps[:], bqk_col[:, 8 + h : 9 + h]
                        )
                    else:
                        nc.scalar.add(kt[:, h, :], kps[:], bqk_col[:, 8 + h : 9 + h])
                # vp chunk: [16, sl] = wvp.T @ x (fp8 DoubleRow); the VS8
                # weight pre-scale is undone for free in the PSUM evacuation.
                # Staged via DRAM as [c, pos, h]: SBUF-source flat-AP gathers
                # with sub-row partition steps fail BIR verification, DRAM
                # APs are unrestricted.
                vp_d = state[("vp_d", b)]
                vps = vp_ps.tile([2 * H, sl], F32, tag="vps")
                for j in range(4):
                    nc.tensor.matmul(
                        vps[:],
                        wvp[:, 2 * j : 2 * j + 2, :],
                        xt[:, 2 * j : 2 * j + 2, ss : ss + sl],
                        start=(j == 0),
                        stop=(j == 3),
                        perf_mode=DR,
                    )
                vpsb = vpsp.tile([2 * H, sl], F32, tag="vpsb")
                nc.vector.tensor_scalar_mul(vpsb[:], vps[:], float(1.0 / VS8))
                rsv = _rowsz(vpsb)
                for c in range(NC):
                    nc.sync.dma_start(
                        out=_flat_ap(
                            vp_d, c * 8 * LP + ss, [[LP, 8], [1, sl]]
                        ),
                        in_=_flat_ap(vpsb, c * rsv, [[2 * rsv, 8], [1, sl]]),
                    )
                state[("kt", b, ci)] = kt

            def attend(b, ci):
                ws, wcnt, ss, sl = CHUNKS[ci]
                qt = state[("qt", b)]
                kt = state.pop(("kt", b, ci))

                # dense per-head scores into one SBUF tile, then a single
                # flat-AP band gather over all heads
                slk = SLK[ci]
                ssb = ssbp.tile([wcnt, 8, slk], F32, tag="ssb")
                for h in range(8):
                    sps = sc_ps.tile([wcnt, slk], F32, tag="sps")
                    nc.tensor.matmul(
                        sps[:], qt[:, h, ws : ws + wcnt], kt[:, h, :],
                        start=True, stop=True,
                    )
                    if h < 5:
                        nc.vector.tensor_copy(ssb[:, h, :], sps[:])
                    else:
                        nc.scalar.copy(ssb[:, h, :], sps[:])
                band = smx.tile([wcnt, 8, T], F32, tag="band")
                rs = _rowsz(ssb)
                nc.sync.dma_start(
                    out=band[:],
                    in_=_flat_ap(ssb, 0, [[rs + 5, wcnt], [slk, 8], [1, T]]),
                )

                # softmax in band domain: subtract per-(w,h) max on gpsimd,
                # one fused Exp on scalar engine, sums via vector reduce
                negmax = smx.tile([wcnt, 8], F32, tag="negmax")
                nc.vector.tensor_reduce(
                    negmax[:], band[:], axis=mybir.AxisListType.X,
                    op=mybir.AluOpType.max, negate=True,
                )
                ebs = smx.tile([wcnt, 8, T], F32, tag="ebs")
                negmax_bc = negmax[:].unsqueeze(2).to_broadcast([wcnt, 8, T])
                nc.gpsimd.tensor_tensor(
                    out=ebs[:], in0=band[:], in1=negmax_bc, op=mybir.AluOpType.add
                )
                eb = smx.tile([wcnt, 8, T], F32, tag="eb")
                nc.scalar.activation(
                    eb[:], ebs[:], mybir.ActivationFunctionType.Exp,
                    bias=0.0, scale=SSCALE,
                )
                sums = smx.tile([wcnt, 8], F32, tag="sums")
                nc.vector.tensor_reduce(
                    sums[:], eb[:], axis=mybir.AxisListType.X,
                    op=mybir.AluOpType.add,
                )
                recip = smx.tile([wcnt, 8], F32, tag="recip")
                nc.vector.reciprocal(recip[:], sums[:])
                attn = smx.tile([wcnt, 8, T], F32, tag="attn")
                recip_bc = recip[:].unsqueeze(2).to_broadcast([wcnt, 8, T])
                nc.gpsimd.tensor_tensor(
                    out=attn[:], in0=eb[:], in1=recip_bc, op=mybir.AluOpType.mult
                )

                # banded value contraction: probs[w, c] = sum_{h,t} attn * vp
                vp_d = state[("vp_d", b)]
                vpb = smx.tile([wcnt, NC, 8, T], F32, tag="vpb")
                for c in range(NC):
                    nc.sync.dma_start(
                        out=vpb[:, c, :, :],
                        in_=_flat_ap(
                            vp_d, 480 * ci + c * 8 * LP,
                            [[5, wcnt], [LP, 8], [1, T]],
                        ),
                    )
                prod = smx.tile([wcnt, NC, 8, T], F32, tag="prod")
                attn_bc = attn[:].unsqueeze(1).to_broadcast([wcnt, NC, 8, T])
                nc.gpsimd.tensor_tensor(
                    out=prod[:], in0=vpb[:], in1=attn_bc, op=mybir.AluOpType.mult
                )
                red1 = smx.tile([wcnt, NC, 8], F32, tag="red1")
                nc.vector.tensor_reduce(
                    red1[:], prod[:], axis=mybir.AxisListType.X,
                    op=mybir.AluOpType.add,
                )
                pb = state[("pb", b)]
                nc.vector.tensor_reduce(
                    pb[0:wcnt, ci, :], red1[:], axis=mybir.AxisListType.X,
                    op=mybir.AluOpType.add,
                )

            def tail_a(b):
                """per-batch tail up to the Exp accumulation (Exp table only,
                which is already resident from the softmax)."""
                pb = state.pop(("pb", b))
                state.pop(("qt", b))
                state.pop(("vp_d", b))
                state.pop(("xt", b))
                pb2 = tailp.tile([96, 5, NC], F32, tag="pb2", name=f"pb2_{b}")
                bias_bc = bias2[0:96, :].unsqueeze(1).to_broadcast([96, 5, NC])
                nc.gpsimd.tensor_tensor(
                    out=pb2[:], in0=pb[:], in1=bias_bc, op=mybir.AluOpType.add
                )
                pmax = tailp.tile([96, 5], F32, tag="pmax", name=f"pmax_{b}")
                nc.vector.tensor_reduce(
                    pmax[:], pb2[:], axis=mybir.AxisListType.X, op=mybir.AluOpType.max
                )
                nmax = perb2.tile([96, 5], F32, tag="nmax")
                nc.gpsimd.tensor_scalar_mul(nmax[:], pmax[:], -1.0)
                sexp = tailp.tile([96, 5], F32, tag="sexp", name=f"sexp_{b}")
                etmp = perb2.tile([96, 5, NC], F32, tag="etmp")
                for wc in range(5):
                    nc.scalar.activation(
                        etmp[:, wc, :], pb2[:, wc, :],
                        mybir.ActivationFunctionType.Exp,
                        bias=nmax[:, wc : wc + 1], scale=1.0,
                        accum_out=sexp[:, wc : wc + 1],
                    )
                state[("pb2", b)] = pb2
                state[("pmax", b)] = pmax
                state[("sexp", b)] = sexp

            def tail_b(b):
                """deferred Ln + final subtract + output scatter (the Ln
                activation table loads once here, after all softmax Exps)."""
                pb2 = state.pop(("pb2", b))
                pmax = state.pop(("pmax", b))
                sexp = state.pop(("sexp", b))
                lns = perb2.tile([96, 5], F32, tag="lns")
                nc.scalar.activation(lns[:], sexp[:], mybir.ActivationFunctionType.Ln)
                otm = perb2.tile([96, 5, NC], F32, tag="otm")
                for wc in range(5):
                    nc.gpsimd.tensor_scalar(
                        out=otm[:, wc, :],
                        in0=pb2[:, wc, :],
                        scalar1=pmax[:, wc : wc + 1],
                        scalar2=lns[:, wc : wc + 1],
                        op0=mybir.AluOpType.subtract,
                        op1=mybir.AluOpType.subtract,
                    )

                rt = _rowsz(otm)
                nc.sync.dma_start(
                    out=bass.AP(
                        tensor=out_d, offset=b * L * NC,
                        ap=[[10, 96], [960, 4], [1, NC]],
                    ),
                    in_=_flat_ap(otm, 0, [[rt, 96], [NC, 4], [1, NC]]),
                )
                nc.sync.dma_start(
                    out=bass.AP(
                        tensor=out_d, offset=b * L * NC + 3840, ap=[[10, 25], [1, NC]]
                    ),
                    in_=_flat_ap(otm, 4 * NC, [[rt, 25], [1, NC]]),
                )

            def start_batch(b):
                load_x(b)
                qproj(b)
                state[("vp_d", b)] = drp.tile(
                    [NC, 8, LP], F32, tag="vp_d", name="vp_d"
                )
                state[("pb", b)] = perb.tile([96, 5, NC], F32, tag="pb", name="pb")

            # ---- software-pipelined emission (2-deep) ----
            iters = [(b, ci) for b in range(BL) for ci in range(5)]
            start_batch(0)
            produce(0, 0)
            for i in range(1, len(iters)):
                b, ci = iters[i]
                if ci == 0:
                    start_batch(b)
                produce(b, ci)
                pb_, pci = iters[i - 1]
                attend(pb_, pci)
                if pci == 4:
                    tail_a(pb_)
            lb, lci = iters[-1]
            attend(lb, lci)
            tail_a(lb)
            for b in range(BL):
                tail_b(b)

    nc.compile()
    return nc


_NC_CACHE = {}


def _get_nc(debug=False):
    if debug not in _NC_CACHE:
        _NC_CACHE[debug] = _build(debug)
    return _NC_CACHE[debug]


def kernel(x, in_proj_w, in_proj_b, out_proj_w, out_proj_b, out_w, out_b, x_len=None,
           _want_perf=False, _debug=False):
    x = np.asarray(x, dtype=np.float32)
    in_proj_w = np.asarray(in_proj_w, dtype=np.float32)
    in_proj_b = np.asarray(in_proj_b, dtype=np.float32)
    out_proj_w = np.asarray(out_proj_w, dtype=np.float32)
    out_proj_b = np.asarray(out_proj_b, dtype=np.float32)
    out_w = np.asarray(out_w, dtype=np.float32)
    out_b = np.asarray(out_b, dtype=np.float32)

    nc = _get_nc(_debug)

    # host-side layout prep; x and q/k weights go to fp8 (e4m3) for
    # DoubleRow matmuls, with weights pre-scaled by WS8 to center the
    # small-magnitude weight distribution in e4m3's normal range.
    xt = np.zeros((B, D, LP), dtype=ml_dtypes.float8_e4m3)
    xt[:, :, :L] = x.transpose(0, 2, 1).astype(ml_dtypes.float8_e4m3)
    xq = np.zeros((B, D, NWQ), dtype=ml_dtypes.float8_e4m3)
    xq[:, :, :NW] = xt[:, :, 5 : 5 * NW + 5 : 5]
    wqkt = np.ascontiguousarray(
        in_proj_w[: 2 * D].T * np.float32(WS8)
    ).astype(ml_dtypes.float8_e4m3)

    # fused rank-2 value path (computed in float64 host-side):
    # wvp[(h,c), :] = (wh @ wo)[c, hslice] @ wv[hslice, :]
    wv = in_proj_w[2 * D :].astype(np.float64)
    bv = in_proj_b[2 * D :].astype(np.float64)
    wo64 = out_proj_w.astype(np.float64)
    wh64 = out_w.astype(np.float64)
    wf = wh64 @ wo64                      # [2, D]
    wvp = np.zeros((2 * H, D), np.float64)
    for h in range(H):
        sl_ = slice(h * HD, (h + 1) * HD)
        wvp[2 * h : 2 * h + 2] = wf[:, sl_] @ wv[sl_, :]
    bc2 = (wh64 @ (wo64 @ bv + out_proj_b.astype(np.float64))
           + out_b.astype(np.float64)).astype(np.float32)
    wvpt = np.ascontiguousarray(wvp.T * VS8).astype(ml_dtypes.float8_e4m3)

    in_maps = []
    for c in range(NCORES):
        in_maps.append({
            "xt": np.ascontiguousarray(xt[c * BL : (c + 1) * BL]),
            "xq": np.ascontiguousarray(xq[c * BL : (c + 1) * BL]),
            "wqkt": wqkt,
            "wvpt": wvpt,
            "bqk": in_proj_b[: 2 * D] * np.float32(WS8),
            "bc2": bc2,
        })

    if _debug:
        kr = run_bass_kernel_spmd(nc, in_maps[:1], core_ids=[0])
        return kr.results[0]
    kr = run_bass_kernel_spmd(
        nc, in_maps, core_ids=list(range(NCORES)), trace=_want_perf
    )
    out = np.concatenate([r["out"] for r in kr.results], axis=0).reshape(-1, NC)
    if _want_perf:
        return out, kr
    return out


# revision 18
# speedup vs baseline: 1.3080x; 1.0487x over previous
"""Trainium2 Bass kernel for nn_ExpWindowAttention (windowed sparse attention).

Strategy: pure data-parallel over batch (32 -> 8 cores x 4 batches).

Key algebraic fusion: only probs = (o @ wo.T + bo) @ wh.T + bh is consumed
(nC=2), so the whole post-softmax pipeline collapses to rank-2 per head:
  probs[w, c] = sum_{h,t} attn[w, h, t] * vp[5w + t, h, c] + BC[c]
  vp = x @ wvp.T,  wvp[(h, c), :] = (wh @ wo)[c, hslice] @ wv[hslice, :]
This eliminates the V projection, attention-times-V, out-projection, and
head GEMMs entirely; vp is a 16-wide projection and the banded contraction
runs on the vector engine.

Q/K/vp projections run in fp8 (e4m3) with DoubleRow matmuls. Weights are
pre-scaled into e4m3's normal range; the score scale folds into the
softmax exp scale and the vp scale into the PSUM evacuation copy.

Scores are computed band-compactly: windows are processed in packs of
4x32, where each 32-window group gets its own 32-column strip of the PE
array (tile_position col-tiling) against its own 176-position k-slice.
The four group matmuls execute concurrently in the array, producing a
[128, 176] per-head score tile whose 11-wide diagonal band is extracted
with flat-AP DMA gathers; softmax runs in the compact band domain.

All host-side tensors are pre-swizzled to the SBUF partition-major layout
so every load is 128 contiguous descriptors (fast HWDGE dispatch), and
startup DMAs are spread across the sync/scalar/gpsimd queues.
"""

import numpy as np
import ml_dtypes

import concourse.bass as bass
from concourse import bacc
import concourse.mybir as mybir
import concourse.tile as tile
from concourse.bass_utils import run_bass_kernel_spmd

F32 = mybir.dt.float32
BF16 = mybir.dt.bfloat16
F8 = mybir.dt.float8e4
DR = mybir.MatmulPerfMode.DoubleRow
WS8 = 128.0              # fp8 weight pre-scale for q/k projections
VS8 = 1024.0             # fp8 weight pre-scale for the vp projection

NCORES = 8
B = 32
BL = B // NCORES          # batches per core
L = 2048
D = 1024
H = 8
HD = 128
W = 5
T = 2 * W + 1             # 11
NW = 409                  # windows per batch
NWQ = 416                 # padded center count (32-mult)
NC = 2
LP = 2176                 # padded position count (17*128)
LK = 2056                 # k/vp positions actually needed (5*408+10+1)
SCALE = float(1.0 / np.sqrt(HD))
SSCALE = float(SCALE / (WS8 * WS8))   # scores carry WS8^2 from fp8 q/k scaling
NEGLOG2 = float(-np.log(2.0))

# non-overlapping K/vp projection segments covering [0, LK)
SEGS = [(0, 512), (512, 512), (1024, 512), (1536, 416), (1952, 104)]
# score chunks: (first window, window count, first position, k-width)
CHUNKS = [(0, 96, 0, 512), (96, 96, 480, 512), (192, 96, 960, 512),
          (288, 96, 1440, 512), (384, 25, 1920, 136)]


def _flat_ap(t, extra_offset, dims):
    """AP over a tile's backing tensor flat element space (partition-major)."""
    return bass.AP(tensor=t.tensor, offset=t.offset + extra_offset,
                   ap=[list(d) for d in dims])


def _rowsz(t):
    """True per-partition stride (elements) of a tile, from its own AP."""
    return int(t[:].ap[0][0])


def _build(debug=False):
    nc = bacc.Bacc(None, target_bir_lowering=False)

    xt_d = nc.declare_dram_parameter("xt", [BL, 128, 8 * LP], F8, isOutput=False)
    xq_d = nc.declare_dram_parameter("xq", [BL, 128, 8 * NWQ], F8, isOutput=False)
    wqk_d = nc.declare_dram_parameter("wqkt", [128, 8 * 2 * D], F8, isOutput=False)
    wvp_d = nc.declare_dram_parameter("wvpt", [128, 8 * 2 * H], F8, isOutput=False)
    bqk_d = nc.declare_dram_parameter("bqk", [128, 16], F32, isOutput=False)
    bc2_d = nc.declare_dram_parameter("bc2", [NC], F32, isOutput=False)
    out_d = nc.declare_dram_parameter("out", [BL, L, NC], F32, isOutput=True)

    with tile.TileContext(nc) as tc:
        import contextlib
        with contextlib.ExitStack() as ctx:
            const = ctx.enter_context(tc.tile_pool(name="const", bufs=1))
            perb = ctx.enter_context(tc.tile_pool(name="perb", bufs=2))
            perb2 = ctx.enter_context(tc.tile_pool(name="perb2", bufs=2))
            tailp = ctx.enter_context(tc.tile_pool(name="tailp", bufs=4))
            xtp = ctx.enter_context(tc.tile_pool(name="xtp", bufs=2))
            ktp = ctx.enter_context(tc.tile_pool(name="ktp", bufs=2))
            vpsp = ctx.enter_context(tc.tile_pool(name="vpsp", bufs=2))
            drp = ctx.enter_context(tc.tile_pool(name="drp", bufs=2, space="DRAM"))
            ssbp = ctx.enter_context(tc.tile_pool(name="ssbp", bufs=3))
            smx = ctx.enter_context(tc.tile_pool(name="smx", bufs=2))
            proj_ps = ctx.enter_context(tc.tile_pool(name="proj_ps", bufs=3, space="PSUM"))
            vp_ps = ctx.enter_context(tc.tile_pool(name="vp_ps", bufs=2, space="PSUM"))
            sc_ps = ctx.enter_context(tc.tile_pool(name="sc_ps", bufs=3, space="PSUM"))

            # ---- resident weights / biases (scalar queue; sync queue kept
            # free for the latency-critical per-pack gathers) ----
            wqk = const.tile([128, 8, 2 * D], F8)
            nc.scalar.dma_start(out=wqk, in_=wqk_d[:, :])
            wvp = const.tile([128, 8, 2 * H], F8)
            nc.scalar.dma_start(out=wvp, in_=wvp_d[:, :])
            bqk_col = const.tile([128, 16], F32)  # [p, proj*8+fc]
            nc.scalar.dma_start(out=bqk_col, in_=bqk_d[:, :])
            bias2 = const.tile([128, NC], F32)    # BC broadcast along partitions
            nc.scalar.dma_start(
                out=bias2, in_=bass.AP(tensor=bc2_d, offset=0, ap=[[0, 128], [1, NC]])
            )
            cfill = const.tile([128, 8], F32)
            nc.vector.memset(cfill, NEGLOG2)

            # all batches' pre-gathered query centers in one resident tile
            xqall = const.tile([128, BL * 8, NWQ], F8)
            for b in range(BL):
                nc.scalar.dma_start(
                    out=xqall[:, b * 8 : (b + 1) * 8, :], in_=xq_d[b]
                )

            state = {}

            def cfill_out(b):
                """Constant log(1/2) for non-window rows (gpsimd queue, fully
                independent of all compute)."""
                nc.gpsimd.dma_start(
                    out=bass.AP(
                        tensor=out_d, offset=b * L * NC + 2,
                        ap=[[1, 1], [10, NW], [1, 8]],
                    ),
                    in_=_flat_ap(cfill, 0, [[_rowsz(cfill), 1], [0, NW], [1, 8]]),
                )
                nc.gpsimd.dma_start(
                    out=bass.AP(
                        tensor=out_d, offset=b * L * NC + 2045 * NC, ap=[[1, 6]]
                    ),
                    in_=cfill[0:1, 0:6],
                )

            def qproj(b):
                """Per-batch Q projection from host-pregathered centers."""
                qt = perb.tile([128, 8, NWQ], BF16, tag="qt")
                for h in range(8):
                    qps = proj_ps.tile([128, NWQ], F32, tag="pps")
                    for j in range(4):
                        nc.tensor.matmul(
                            qps[:],
                            wqk[:, 2 * j : 2 * j + 2, h * 128 : h * 128 + 128],
                            xqall[:, b * 8 + 2 * j : b * 8 + 2 * j + 2, :],
                            start=(j == 0),
                            stop=(j == 3),
                            perf_mode=DR,
                        )
                    if h % 2 == 0:
                        nc.vector.tensor_scalar_add(
                            qt[:, h, :], qps[:], bqk_col[:, h : h + 1]
                        )
                    else:
                        nc.scalar.add(qt[:, h, :], qps[:], bqk_col[:, h : h + 1])
                state[("qt", b)] = qt

            def load_x(b):
                """Whole-batch transposed-x load (one large contiguous DMA)."""
                xt = xtp.tile([128, 8, LP], F8, tag="xt")
                nc.scalar.dma_start(out=xt, in_=xt_d[b])
                state[("xt", b)] = xt

            def produce(b, si):
                """K projection + vp (rank-2 value path) for one (batch, seg)."""
                ss, sl = SEGS[si]
                xt = state[("xt", b)]
                kt = state[("kt", b)]
                for h in range(8):
                    kps = proj_ps.tile([128, sl], F32, tag="pps")
                    for j in range(4):
                        nc.tensor.matmul(
                            kps[:],
                            wqk[:, 2 * j : 2 * j + 2, D + h * 128 : D + h * 128 + 128],
                            xt[:, 2 * j : 2 * j + 2, ss : ss + sl],
                            start=(j == 0),
                            stop=(j == 3),
                            perf_mode=DR,
                        )
                    if h < 3:
                        nc.vector.tensor_scalar_add(
                            kt[:, h, ss : ss + sl], kps[:], bqk_col[:, 8 + h : 9 + h]
                        )
                    else:
                        nc.scalar.add(
                            kt[:, h, ss : ss + sl], kps[:], bqk_col[:, 8 + h : 9 + h]
                        )
                # vp seg: [16, sl] = wvp.T @ x (fp8 DoubleRow); the VS8 weight
                # pre-scale is undone for free in the PSUM evacuation. Staged
                # via DRAM as [c, pos, h]: SBUF-source flat-AP gathers with
                # sub-row partition steps fail BIR verification, DRAM APs are
                # unrestricted.
                vp_d = state[("vp_d", b)]
                vps = vp_ps.tile([2 * H, sl], F32, tag="vps")
                for j in range(4):
                    nc.tensor.matmul(
                        vps[:],
                        wvp[:, 2 * j : 2 * j + 2, :],
                        xt[:, 2 * j : 2 * j + 2, ss : ss + sl],
                        start=(j == 0),
                        stop=(j == 3),
                        perf_mode=DR,
                    )
                vpsb = vpsp.tile([2 * H, sl], F32, tag="vpsb")
                nc.vector.tensor_scalar_mul(vpsb[:], vps[:], float(1.0 / VS8))
                rsv = _rowsz(vpsb)
                for c in range(NC):
                    nc.sync.dma_start(
                        out=_flat_ap(vp_d, c * 8 * LP + ss, [[LP, 8], [1, sl]]),
                        in_=_flat_ap(vpsb, c * rsv, [[2 * rsv, 8], [1, sl]]),
                    )

            def attend(b, ci):
                """Dense per-head scores for one chunk of windows, single
                flat-AP band gather, softmax, banded value contraction."""
                wst, wcnt, pst, kw = CHUNKS[ci]
                qt = state[("qt", b)]
                kt = state[("kt", b)]

                ssb = ssbp.tile([wcnt, 8, kw], F32, tag="ssb")
                for h in range(8):
                    sps = sc_ps.tile([wcnt, kw], F32, tag="sps")
                    nc.tensor.matmul(
                        sps[:], qt[:, h, wst : wst + wcnt],
                        kt[:, h, pst : pst + kw],
                        start=True, stop=True,
                    )
                    if h < 4:
                        nc.vector.tensor_copy(ssb[:, h, :], sps[:])
                    else:
                        nc.scalar.copy(ssb[:, h, :], sps[:])

                # single flat-AP band gather over all heads
                band = smx.tile([wcnt, 8, T], F32, tag="band")
                rs = _rowsz(ssb)
                nc.sync.dma_start(
                    out=band[:],
                    in_=_flat_ap(ssb, 0, [[rs + 5, wcnt], [kw, 8], [1, T]]),
                )

                # softmax in band domain
                negmax = smx.tile([wcnt, 8], F32, tag="negmax")
                nc.vector.tensor_reduce(
                    negmax[:], band[:], axis=mybir.AxisListType.X,
                    op=mybir.AluOpType.max, negate=True,
                )
                ebs = smx.tile([wcnt, 8, T], F32, tag="ebs")
                negmax_bc = negmax[:].unsqueeze(2).to_broadcast([wcnt, 8, T])
                nc.gpsimd.tensor_tensor(
                    out=ebs[:], in0=band[:], in1=negmax_bc,
                    op=mybir.AluOpType.add,
                )
                eb = smx.tile([wcnt, 8, T], F32, tag="eb")
                nc.scalar.activation(
                    eb[:], ebs[:], mybir.ActivationFunctionType.Exp,
                    bias=0.0, scale=SSCALE,
                )
                sums = smx.tile([wcnt, 8], F32, tag="sums")
                nc.vector.tensor_reduce(
                    sums[:], eb[:], axis=mybir.AxisListType.X,
                    op=mybir.AluOpType.add,
                )
                recip = smx.tile([wcnt, 8], F32, tag="recip")
                nc.vector.reciprocal(recip[:], sums[:])
                attn = smx.tile([wcnt, 8, T], F32, tag="attn")
                recip_bc = recip[:].unsqueeze(2).to_broadcast([wcnt, 8, T])
                nc.gpsimd.tensor_tensor(
                    out=attn[:], in0=eb[:], in1=recip_bc,
                    op=mybir.AluOpType.mult,
                )

                # banded value contraction: probs[w, c] = sum_{h,t} attn * vp
                vp_d = state[("vp_d", b)]
                vpb = smx.tile([wcnt, NC, 8, T], F32, tag="vpb")
                for c in range(NC):
                    nc.sync.dma_start(
                        out=vpb[:, c, :, :],
                        in_=_flat_ap(
                            vp_d, 5 * wst + c * 8 * LP,
                            [[5, wcnt], [LP, 8], [1, T]],
                        ),
                    )
                prod = smx.tile([wcnt, NC, 8, T], F32, tag="prod")
                attn_bc = attn[:].unsqueeze(1).to_broadcast([wcnt, NC, 8, T])
                nc.gpsimd.tensor_tensor(
                    out=prod[:], in0=vpb[:], in1=attn_bc,
                    op=mybir.AluOpType.mult,
                )
                red1 = smx.tile([wcnt, NC, 8], F32, tag="red1")
                nc.vector.tensor_reduce(
                    red1[:], prod[:], axis=mybir.AxisListType.X,
                    op=mybir.AluOpType.add,
                )
                pb = state[("pb", b)]
                nc.vector.tensor_reduce(
                    pb[0:wcnt, ci, :], red1[:],
                    axis=mybir.AxisListType.X, op=mybir.AluOpType.add,
                )

            def tail_a(b):
                """per-batch tail up to the Exp accumulation (Exp table only,
                which is already resident from the softmax)."""
                pb = state.pop(("pb", b))
                state.pop(("qt", b))
                state.pop(("vp_d", b))
                state.pop(("xt", b))
                state.pop(("kt", b))
                pb2 = tailp.tile([96, 5, NC], F32, tag="pb2", name=f"pb2_{b}")
                bias_bc = bias2[0:96, :].unsqueeze(1).to_broadcast([96, 5, NC])
                nc.gpsimd.tensor_tensor(
                    out=pb2[:], in0=pb[:], in1=bias_bc, op=mybir.AluOpType.add
                )
                pmax = tailp.tile([96, 5], F32, tag="pmax", name=f"pmax_{b}")
                nc.vector.tensor_reduce(
                    pmax[:], pb2[:], axis=mybir.AxisListType.X, op=mybir.AluOpType.max
                )
                nmax = perb2.tile([96, 5], F32, tag="nmax")
                nc.gpsimd.tensor_scalar_mul(nmax[:], pmax[:], -1.0)
                sexp = tailp.tile([96, 5], F32, tag="sexp", name=f"sexp_{b}")
                etmp = perb2.tile([96, 5, NC], F32, tag="etmp")
                for p in range(5):
                    nc.scalar.activation(
                        etmp[:, p, :], pb2[:, p, :],
                        mybir.ActivationFunctionType.Exp,
                        bias=nmax[:, p : p + 1], scale=1.0,
                        accum_out=sexp[:, p : p + 1],
                    )
                state[("pb2", b)] = pb2
                state[("pmax", b)] = pmax
                state[("sexp", b)] = sexp

            def tail_b(b):
                """deferred Ln + final subtract + output scatter (the Ln
                activation table loads once here, after all softmax Exps)."""
                pb2 = state.pop(("pb2", b))
                pmax = state.pop(("pmax", b))
                sexp = state.pop(("sexp", b))
                lns = perb2.tile([96, 5], F32, tag="lns")
                nc.scalar.activation(lns[:], sexp[:], mybir.ActivationFunctionType.Ln)
                otm = perb2.tile([96, 5, NC], F32, tag="otm")
                for p in range(5):
                    nc.gpsimd.tensor_scalar(
                        out=otm[:, p, :],
                        in0=pb2[:, p, :],
                        scalar1=pmax[:, p : p + 1],
                        scalar2=lns[:, p : p + 1],
                        op0=mybir.AluOpType.subtract,
                        op1=mybir.AluOpType.subtract,
                    )

                rt = _rowsz(otm)
                nc.sync.dma_start(
                    out=bass.AP(
                        tensor=out_d, offset=b * L * NC,
                        ap=[[10, 96], [960, 4], [1, NC]],
                    ),
                    in_=_flat_ap(otm, 0, [[rt, 96], [NC, 4], [1, NC]]),
                )
                nc.sync.dma_start(
                    out=bass.AP(
                        tensor=out_d, offset=b * L * NC + 3840, ap=[[10, 25], [1, NC]]
                    ),
                    in_=_flat_ap(otm, 4 * NC, [[rt, 25], [1, NC]]),
                )

            def start_batch(b):
                load_x(b)
                qproj(b)
                state[("kt", b)] = ktp.tile([128, 8, LK], BF16, tag="kt", name="kt")
                state[("vp_d", b)] = drp.tile(
                    [NC, 8, LP], F32, tag="vp_d", name="vp_d"
                )
                state[("pb", b)] = perb.tile([96, 5, NC], F32, tag="pb", name="pb")
                cfill_out(b)

            # ---- software-pipelined emission ----
            # pack p needs segs {p, p+1} (p<3) / {3, 4} (p=3)
            start_batch(0)
            for b in range(BL):
                produce(b, 0)
                produce(b, 1)
                attend(b, 0)
                if b + 1 < BL:
                    start_batch(b + 1)
                produce(b, 2)
                attend(b, 1)
                produce(b, 3)
                attend(b, 2)
                produce(b, 4)
                attend(b, 3)
                attend(b, 4)
                tail_a(b)
            for b in range(BL):
                tail_b(b)

    nc.compile()
    return nc


_NC_CACHE = {}


def _get_nc(debug=False):
    if debug not in _NC_CACHE:
        _NC_CACHE[debug] = _build(debug)
    return _NC_CACHE[debug]


def _swz(a):
    """[X*128, C] -> [128, X*C] partition-major pre-swizzle (X=8 groups)."""
    x, c = a.shape[0] // 128, a.shape[1]
    return np.ascontiguousarray(
        a.reshape(x, 128, c).transpose(1, 0, 2).reshape(128, x * c)
    )


def kernel(x, in_proj_w, in_proj_b, out_proj_w, out_proj_b, out_w, out_b, x_len=None,
           _want_perf=False, _debug=False):
    x = np.asarray(x, dtype=np.float32)
    in_proj_w = np.asarray(in_proj_w, dtype=np.float32)
    in_proj_b = np.asarray(in_proj_b, dtype=np.float32)
    out_proj_w = np.asarray(out_proj_w, dtype=np.float32)
    out_proj_b = np.asarray(out_proj_b, dtype=np.float32)
    out_w = np.asarray(out_w, dtype=np.float32)
    out_b = np.asarray(out_b, dtype=np.float32)

    nc = _get_nc(_debug)

    # host-side layout prep; x and q/k weights go to fp8 (e4m3) for
    # DoubleRow matmuls, with weights pre-scaled by WS8 to center the
    # small-magnitude weight distribution in e4m3's normal range. All
    # device tensors are pre-swizzled to SBUF partition-major layout.
    xt = np.zeros((B, D, LP), dtype=ml_dtypes.float8_e4m3)
    xt[:, :, :L] = x.transpose(0, 2, 1).astype(ml_dtypes.float8_e4m3)
    xq = np.ascontiguousarray(xt[:, :, 5 : 5 * NW + 5 : 5])
    xqp = np.zeros((B, D, NWQ), dtype=ml_dtypes.float8_e4m3)
    xqp[:, :, :NW] = xq
    xt8 = np.stack([_swz(xt[i]) for i in range(B)])
    xq8 = np.stack([_swz(xqp[i]) for i in range(B)])
    wqkt = _swz(
        (in_proj_w[: 2 * D].T * np.float32(WS8)).astype(ml_dtypes.float8_e4m3)
    )

    # fused rank-2 value path (computed in float64 host-side):
    # wvp[(h,c), :] = (wh @ wo)[c, hslice] @ wv[hslice, :]
    wv = in_proj_w[2 * D :].astype(np.float64)
    bv = in_proj_b[2 * D :].astype(np.float64)
    wo64 = out_proj_w.astype(np.float64)
    wh64 = out_w.astype(np.float64)
    wf = wh64 @ wo64                      # [2, D]
    wvp = np.zeros((2 * H, D), np.float64)
    for h in range(H):
        sl_ = slice(h * HD, (h + 1) * HD)
        wvp[2 * h : 2 * h + 2] = wf[:, sl_] @ wv[sl_, :]
    bc2 = (wh64 @ (wo64 @ bv + out_proj_b.astype(np.float64))
           + out_b.astype(np.float64)).astype(np.float32)
    wvpt = _swz((wvp.T * VS8).astype(ml_dtypes.float8_e4m3))
    bqk8 = _swz(
        (in_proj_b[: 2 * D] * np.float32(WS8)).reshape(2 * D, 1)
    ).reshape(128, 16)

    in_maps = []
    for c in range(NCORES):
        in_maps.append({
            "xt": np.ascontiguousarray(xt8[c * BL : (c + 1) * BL]),
            "xq": np.ascontiguousarray(xq8[c * BL : (c + 1) * BL]),
            "wqkt": wqkt,
            "wvpt": wvpt,
            "bqk": bqk8,
            "bc2": bc2,
        })

    if _debug:
        kr = run_bass_kernel_spmd(nc, in_maps[:1], core_ids=[0])
        return kr.results[0]
    kr = run_bass_kernel_spmd(
        nc, in_maps, core_ids=list(range(NCORES)), trace=_want_perf
    )
    out = np.concatenate([r["out"] for r in kr.results], axis=0).reshape(-1, NC)
    if _want_perf:
        return out, kr
    return out


# revision 19
# speedup vs baseline: 1.3448x; 1.0281x over previous
"""Trainium2 Bass kernel for nn_ExpWindowAttention (windowed sparse attention).

Strategy: pure data-parallel over batch (32 -> 8 cores x 4 batches).

Key algebraic fusion: only probs = (o @ wo.T + bo) @ wh.T + bh is consumed
(nC=2), so the whole post-softmax pipeline collapses to rank-2 per head:
  probs[w, c] = sum_{h,t} attn[w, h, t] * vp[5w + t, h, c] + BC[c]
  vp = x @ wvp.T,  wvp[(h, c), :] = (wh @ wo)[c, hslice] @ wv[hslice, :]
This eliminates the V projection, attention-times-V, out-projection, and
head GEMMs entirely; vp is a 16-wide projection and the banded contraction
runs on the vector engine.

Q/K/vp projections run in fp8 (e4m3) with DoubleRow matmuls. Weights are
pre-scaled into e4m3's normal range; the score scale folds into the
softmax exp scale and the vp scale into the PSUM evacuation copy.

Scores are computed band-compactly: windows are processed in packs of
4x32, where each 32-window group gets its own 32-column strip of the PE
array (tile_position col-tiling) against its own 176-position k-slice.
The four group matmuls execute concurrently in the array, producing a
[128, 176] per-head score tile whose 11-wide diagonal band is extracted
with flat-AP DMA gathers; softmax runs in the compact band domain.

All host-side tensors are pre-swizzled to the SBUF partition-major layout
so every load is 128 contiguous descriptors (fast HWDGE dispatch), and
startup DMAs are spread across the sync/scalar/gpsimd queues.
"""

import numpy as np
import ml_dtypes

import concourse.bass as bass
from concourse import bacc
import concourse.mybir as mybir
import concourse.tile as tile
from concourse.bass_utils import run_bass_kernel_spmd

F32 = mybir.dt.float32
BF16 = mybir.dt.bfloat16
F8 = mybir.dt.float8e4
DR = mybir.MatmulPerfMode.DoubleRow
WS8 = 128.0              # fp8 weight pre-scale for q/k projections
VS8 = 1024.0             # fp8 weight pre-scale for the vp projection

NCORES = 8
B = 32
BL = B // NCORES          # batches per core
L = 2048
D = 1024
H = 8
HD = 128
W = 5
T = 2 * W + 1             # 11
NW = 409                  # windows per batch
NWQ = 416                 # padded center count (32-mult)
NC = 2
LP = 2176                 # padded position count (17*128)
LK = 2056                 # k/vp positions actually needed (5*408+10+1)
SCALE = float(1.0 / np.sqrt(HD))
SSCALE = float(SCALE / (WS8 * WS8))   # scores carry WS8^2 from fp8 q/k scaling
NEGLOG2 = float(-np.log(2.0))

# non-overlapping K/vp projection segments covering [0, LK)
SEGS = [(0, 512), (512, 512), (1024, 512), (1536, 416), (1952, 104)]
# score chunks: (first window, window count, first position, k-width)
CHUNKS = [(0, 96, 0, 512), (96, 96, 480, 512), (192, 96, 960, 512),
          (288, 96, 1440, 512), (384, 25, 1920, 136)]


def _flat_ap(t, extra_offset, dims):
    """AP over a tile's backing tensor flat element space (partition-major)."""
    return bass.AP(tensor=t.tensor, offset=t.offset + extra_offset,
                   ap=[list(d) for d in dims])


def _rowsz(t):
    """True per-partition stride (elements) of a tile, from its own AP."""
    return int(t[:].ap[0][0])


def _build(debug=False):
    nc = bacc.Bacc(None, target_bir_lowering=False)

    xt_d = nc.declare_dram_parameter("xt", [BL, 128, 8 * LP], F8, isOutput=False)
    xq_d = nc.declare_dram_parameter("xq", [BL, 128, 8 * NWQ], F8, isOutput=False)
    wqk_d = nc.declare_dram_parameter("wqkt", [128, 8 * 2 * D], F8, isOutput=False)
    wvp_d = nc.declare_dram_parameter("wvpt", [128, 8 * 2 * H], F8, isOutput=False)
    bqk_d = nc.declare_dram_parameter("bqk", [128, 16], F32, isOutput=False)
    bc2_d = nc.declare_dram_parameter("bc2", [NC], F32, isOutput=False)
    out_d = nc.declare_dram_parameter("out", [BL, L, NC], F32, isOutput=True)

    with tile.TileContext(nc) as tc:
        import contextlib
        with contextlib.ExitStack() as ctx:
            const = ctx.enter_context(tc.tile_pool(name="const", bufs=1))
            perb = ctx.enter_context(tc.tile_pool(name="perb", bufs=2))
            perb2 = ctx.enter_context(tc.tile_pool(name="perb2", bufs=2))
            tailp = ctx.enter_context(tc.tile_pool(name="tailp", bufs=4))
            xtp = ctx.enter_context(tc.tile_pool(name="xtp", bufs=2))
            ktp = ctx.enter_context(tc.tile_pool(name="ktp", bufs=2))
            vpsp = ctx.enter_context(tc.tile_pool(name="vpsp", bufs=2))
            drp = ctx.enter_context(tc.tile_pool(name="drp", bufs=2, space="DRAM"))
            ssbp = ctx.enter_context(tc.tile_pool(name="ssbp", bufs=3))
            smx = ctx.enter_context(tc.tile_pool(name="smx", bufs=2))
            proj_ps = ctx.enter_context(tc.tile_pool(name="proj_ps", bufs=3, space="PSUM"))
            vp_ps = ctx.enter_context(tc.tile_pool(name="vp_ps", bufs=2, space="PSUM"))
            sc_ps = ctx.enter_context(tc.tile_pool(name="sc_ps", bufs=3, space="PSUM"))

            # ---- resident weights / biases (scalar queue; sync queue kept
            # free for the latency-critical per-pack gathers) ----
            xqall = const.tile([128, BL * 8, NWQ], F8)
            nc.scalar.dma_start(out=xqall[:, 0:8, :], in_=xq_d[0])
            wqk = const.tile([128, 8, 2 * D], F8)
            for j in range(4):
                nc.scalar.dma_start(
                    out=wqk[:, 2 * j : 2 * j + 2, :],
                    in_=wqk_d[:, 2 * j * 2 * D : (2 * j + 2) * 2 * D],
                )
            wvp = const.tile([128, 8, 2 * H], F8)
            nc.scalar.dma_start(out=wvp, in_=wvp_d[:, :])
            bqk_col = const.tile([128, 16], F32)  # [p, proj*8+fc]
            nc.scalar.dma_start(out=bqk_col, in_=bqk_d[:, :])
            bias2 = const.tile([128, NC], F32)    # BC broadcast along partitions
            nc.scalar.dma_start(
                out=bias2, in_=bass.AP(tensor=bc2_d, offset=0, ap=[[0, 128], [1, NC]])
            )
            cfill = const.tile([128, 8], F32)
            nc.vector.memset(cfill, NEGLOG2)
            xqall_rest_loaded = [False]

            def load_xq_rest():
                for b in range(1, BL):
                    nc.scalar.dma_start(
                        out=xqall[:, b * 8 : (b + 1) * 8, :], in_=xq_d[b]
                    )

            state = {}

            def cfill_out(b):
                """Constant log(1/2) for non-window rows (gpsimd queue, fully
                independent of all compute)."""
                nc.gpsimd.dma_start(
                    out=bass.AP(
                        tensor=out_d, offset=b * L * NC + 2,
                        ap=[[1, 1], [10, NW], [1, 8]],
                    ),
                    in_=_flat_ap(cfill, 0, [[_rowsz(cfill), 1], [0, NW], [1, 8]]),
                )
                nc.gpsimd.dma_start(
                    out=bass.AP(
                        tensor=out_d, offset=b * L * NC + 2045 * NC, ap=[[1, 6]]
                    ),
                    in_=cfill[0:1, 0:6],
                )

            def qproj(b):
                """Per-batch Q projection from host-pregathered centers."""
                qt = perb.tile([128, 8, NWQ], BF16, tag="qt")
                for h in range(8):
                    qps = proj_ps.tile([128, NWQ], F32, tag="pps")
                    for j in range(4):
                        nc.tensor.matmul(
                            qps[:],
                            wqk[:, 2 * j : 2 * j + 2, h * 128 : h * 128 + 128],
                            xqall[:, b * 8 + 2 * j : b * 8 + 2 * j + 2, :],
                            start=(j == 0),
                            stop=(j == 3),
                            perf_mode=DR,
                        )
                    if h % 2 == 0:
                        nc.vector.tensor_scalar_add(
                            qt[:, h, :], qps[:], bqk_col[:, h : h + 1]
                        )
                    else:
                        nc.scalar.add(qt[:, h, :], qps[:], bqk_col[:, h : h + 1])
                state[("qt", b)] = qt

            def load_x(b):
                """Whole-batch transposed-x load (one large contiguous DMA)."""
                xt = xtp.tile([128, 8, LP], F8, tag="xt")
                nc.sync.dma_start(out=xt, in_=xt_d[b])
                state[("xt", b)] = xt

            def produce(b, si):
                """K projection + vp (rank-2 value path) for one (batch, seg)."""
                ss, sl = SEGS[si]
                xt = state[("xt", b)]
                kt = state[("kt", b)]
                for h in range(8):
                    kps = proj_ps.tile([128, sl], F32, tag="pps")
                    for j in range(4):
                        nc.tensor.matmul(
                            kps[:],
                            wqk[:, 2 * j : 2 * j + 2, D + h * 128 : D + h * 128 + 128],
                            xt[:, 2 * j : 2 * j + 2, ss : ss + sl],
                            start=(j == 0),
                            stop=(j == 3),
                            perf_mode=DR,
                        )
                    if h < 4:
                        nc.vector.tensor_scalar_add(
                            kt[:, h, ss : ss + sl], kps[:], bqk_col[:, 8 + h : 9 + h]
                        )
                    else:
                        nc.scalar.add(
                            kt[:, h, ss : ss + sl], kps[:], bqk_col[:, 8 + h : 9 + h]
                        )
                # vp seg: [16, sl] = wvp.T @ x (fp8 DoubleRow); the VS8 weight
                # pre-scale is undone for free in the PSUM evacuation. Staged
                # via DRAM as [c, pos, h]: SBUF-source flat-AP gathers with
                # sub-row partition steps fail BIR verification, DRAM APs are
                # unrestricted.
                vp_d = state[("vp_d", b)]
                vps = vp_ps.tile([2 * H, sl], F32, tag="vps")
                for j in range(4):
                    nc.tensor.matmul(
                        vps[:],
                        wvp[:, 2 * j : 2 * j + 2, :],
                        xt[:, 2 * j : 2 * j + 2, ss : ss + sl],
                        start=(j == 0),
                        stop=(j == 3),
                        perf_mode=DR,
                    )
                vpsb = vpsp.tile([2 * H, sl], F32, tag="vpsb")
                nc.vector.tensor_scalar_mul(vpsb[:], vps[:], float(1.0 / VS8))
                rsv = _rowsz(vpsb)
                for c in range(NC):
                    nc.sync.dma_start(
                        out=_flat_ap(vp_d, c * 8 * LP + ss, [[LP, 8], [1, sl]]),
                        in_=_flat_ap(vpsb, c * rsv, [[2 * rsv, 8], [1, sl]]),
                    )

            def attend(b, ci):
                """Dense per-head scores for one chunk of windows, single
                flat-AP band gather, softmax, banded value contraction."""
                wst, wcnt, pst, kw = CHUNKS[ci]
                qt = state[("qt", b)]
                kt = state[("kt", b)]

                ssb = ssbp.tile([wcnt, 8, kw], F32, tag="ssb")
                for h in range(8):
                    sps = sc_ps.tile([wcnt, kw], F32, tag="sps")
                    nc.tensor.matmul(
                        sps[:], qt[:, h, wst : wst + wcnt],
                        kt[:, h, pst : pst + kw],
                        start=True, stop=True,
                    )
                    if h < 4:
                        nc.vector.tensor_copy(ssb[:, h, :], sps[:])
                    else:
                        nc.scalar.copy(ssb[:, h, :], sps[:])

                # single flat-AP band gather over all heads
                band = smx.tile([wcnt, 8, T], F32, tag="band")
                rs = _rowsz(ssb)
                nc.sync.dma_start(
                    out=band[:],
                    in_=_flat_ap(ssb, 0, [[rs + 5, wcnt], [kw, 8], [1, T]]),
                )

                # softmax in band domain
                negmax = smx.tile([wcnt, 8], F32, tag="negmax")
                nc.vector.tensor_reduce(
                    negmax[:], band[:], axis=mybir.AxisListType.X,
                    op=mybir.AluOpType.max, negate=True,
                )
                ebs = smx.tile([wcnt, 8, T], F32, tag="ebs")
                negmax_bc = negmax[:].unsqueeze(2).to_broadcast([wcnt, 8, T])
                nc.gpsimd.tensor_tensor(
                    out=ebs[:], in0=band[:], in1=negmax_bc,
                    op=mybir.AluOpType.add,
                )
                eb = smx.tile([wcnt, 8, T], F32, tag="eb")
                nc.scalar.activation(
                    eb[:], ebs[:], mybir.ActivationFunctionType.Exp,
                    bias=0.0, scale=SSCALE,
                )
                sums = smx.tile([wcnt, 8], F32, tag="sums")
                nc.vector.tensor_reduce(
                    sums[:], eb[:], axis=mybir.AxisListType.X,
                    op=mybir.AluOpType.add,
                )
                recip = smx.tile([wcnt, 8], F32, tag="recip")
                nc.vector.reciprocal(recip[:], sums[:])
                attn = smx.tile([wcnt, 8, T], F32, tag="attn")
                recip_bc = recip[:].unsqueeze(2).to_broadcast([wcnt, 8, T])
                nc.gpsimd.tensor_tensor(
                    out=attn[:], in0=eb[:], in1=recip_bc,
                    op=mybir.AluOpType.mult,
                )

                # banded value contraction: probs[w, c] = sum_{h,t} attn * vp
                vp_d = state[("vp_d", b)]
                vpb = smx.tile([wcnt, NC, 8, T], F32, tag="vpb")
                for c in range(NC):
                    nc.sync.dma_start(
                        out=vpb[:, c, :, :],
                        in_=_flat_ap(
                            vp_d, 5 * wst + c * 8 * LP,
                            [[5, wcnt], [LP, 8], [1, T]],
                        ),
                    )
                prod = smx.tile([wcnt, NC, 8, T], F32, tag="prod")
                attn_bc = attn[:].unsqueeze(1).to_broadcast([wcnt, NC, 8, T])
                nc.gpsimd.tensor_tensor(
                    out=prod[:], in0=vpb[:], in1=attn_bc,
                    op=mybir.AluOpType.mult,
                )
                red1 = smx.tile([wcnt, NC, 8], F32, tag="red1")
                nc.vector.tensor_reduce(
                    red1[:], prod[:], axis=mybir.AxisListType.X,
                    op=mybir.AluOpType.add,
                )
                pb = state[("pb", b)]
                nc.vector.tensor_reduce(
                    pb[0:wcnt, ci, :], red1[:],
                    axis=mybir.AxisListType.X, op=mybir.AluOpType.add,
                )

            def tail_a(b):
                """per-batch tail up to the Exp accumulation (Exp table only,
                which is already resident from the softmax)."""
                pb = state.pop(("pb", b))
                state.pop(("qt", b))
                state.pop(("vp_d", b))
                state.pop(("xt", b))
                state.pop(("kt", b))
                pb2 = tailp.tile([96, 5, NC], F32, tag="pb2", name=f"pb2_{b}")
                bias_bc = bias2[0:96, :].unsqueeze(1).to_broadcast([96, 5, NC])
                nc.gpsimd.tensor_tensor(
                    out=pb2[:], in0=pb[:], in1=bias_bc, op=mybir.AluOpType.add
                )
                pmax = tailp.tile([96, 5], F32, tag="pmax", name=f"pmax_{b}")
                nc.vector.tensor_reduce(
                    pmax[:], pb2[:], axis=mybir.AxisListType.X, op=mybir.AluOpType.max
                )
                nmax = perb2.tile([96, 5], F32, tag="nmax")
                nc.gpsimd.tensor_scalar_mul(nmax[:], pmax[:], -1.0)
                sexp = tailp.tile([96, 5], F32, tag="sexp", name=f"sexp_{b}")
                etmp = perb2.tile([96, 5, NC], F32, tag="etmp")
                for p in range(5):
                    nc.scalar.activation(
                        etmp[:, p, :], pb2[:, p, :],
                        mybir.ActivationFunctionType.Exp,
                        bias=nmax[:, p : p + 1], scale=1.0,
                        accum_out=sexp[:, p : p + 1],
                    )
                state[("pb2", b)] = pb2
                state[("pmax", b)] = pmax
                state[("sexp", b)] = sexp

            def tail_b(b):
                """deferred Ln + final subtract + output scatter (the Ln
                activation table loads once here, after all softmax Exps)."""
                pb2 = state.pop(("pb2", b))
                pmax = state.pop(("pmax", b))
                sexp = state.pop(("sexp", b))
                lns = perb2.tile([96, 5], F32, tag="lns")
                nc.scalar.activation(lns[:], sexp[:], mybir.ActivationFunctionType.Ln)
                otm = perb2.tile([96, 5, NC], F32, tag="otm")
                for p in range(5):
                    nc.gpsimd.tensor_scalar(
                        out=otm[:, p, :],
                        in0=pb2[:, p, :],
                        scalar1=pmax[:, p : p + 1],
                        scalar2=lns[:, p : p + 1],
                        op0=mybir.AluOpType.subtract,
                        op1=mybir.AluOpType.subtract,
                    )

                rt = _rowsz(otm)
                nc.sync.dma_start(
                    out=bass.AP(
                        tensor=out_d, offset=b * L * NC,
                        ap=[[10, 96], [960, 4], [1, NC]],
                    ),
                    in_=_flat_ap(otm, 0, [[rt, 96], [NC, 4], [1, NC]]),
                )
                nc.sync.dma_start(
                    out=bass.AP(
                        tensor=out_d, offset=b * L * NC + 3840, ap=[[10, 25], [1, NC]]
                    ),
                    in_=_flat_ap(otm, 4 * NC, [[rt, 25], [1, NC]]),
                )

            def start_batch(b):
                load_x(b)
                qproj(b)
                state[("kt", b)] = ktp.tile([128, 8, LK], BF16, tag="kt", name="kt")
                state[("vp_d", b)] = drp.tile(
                    [NC, 8, LP], F32, tag="vp_d", name="vp_d"
                )
                state[("pb", b)] = perb.tile([96, 5, NC], F32, tag="pb", name="pb")
                cfill_out(b)

            # ---- software-pipelined emission ----
            # pack p needs segs {p, p+1} (p<3) / {3, 4} (p=3)
            start_batch(0)
            for b in range(BL):
                produce(b, 0)
                produce(b, 1)
                if b == 0:
                    load_xq_rest()
                attend(b, 0)
                if b + 1 < BL:
                    start_batch(b + 1)
                produce(b, 2)
                attend(b, 1)
                produce(b, 3)
                attend(b, 2)
                produce(b, 4)
                attend(b, 4)
                attend(b, 3)
                tail_a(b)
            for b in range(BL):
                tail_b(b)

    nc.compile()
    return nc


_NC_CACHE = {}


def _get_nc(debug=False):
    if debug not in _NC_CACHE:
        _NC_CACHE[debug] = _build(debug)
    return _NC_CACHE[debug]


def _swz(a):
    """[X*128, C] -> [128, X*C] partition-major pre-swizzle (X=8 groups)."""
    x, c = a.shape[0] // 128, a.shape[1]
    return np.ascontiguousarray(
        a.reshape(x, 128, c).transpose(1, 0, 2).reshape(128, x * c)
    )


def kernel(x, in_proj_w, in_proj_b, out_proj_w, out_proj_b, out_w, out_b, x_len=None,
           _want_perf=False, _debug=False):
    x = np.asarray(x, dtype=np.float32)
    in_proj_w = np.asarray(in_proj_w, dtype=np.float32)
    in_proj_b = np.asarray(in_proj_b, dtype=np.float32)
    out_proj_w = np.asarray(out_proj_w, dtype=np.float32)
    out_proj_b = np.asarray(out_proj_b, dtype=np.float32)
    out_w = np.asarray(out_w, dtype=np.float32)
    out_b = np.asarray(out_b, dtype=np.float32)

    nc = _get_nc(_debug)

    # host-side layout prep; x and q/k weights go to fp8 (e4m3) for
    # DoubleRow matmuls, with weights pre-scaled by WS8 to center the
    # small-magnitude weight distribution in e4m3's normal range. All
    # device tensors are pre-swizzled to SBUF partition-major layout.
    xt = np.zeros((B, D, LP), dtype=ml_dtypes.float8_e4m3)
    xt[:, :, :L] = x.transpose(0, 2, 1).astype(ml_dtypes.float8_e4m3)
    xq = np.ascontiguousarray(xt[:, :, 5 : 5 * NW + 5 : 5])
    xqp = np.zeros((B, D, NWQ), dtype=ml_dtypes.float8_e4m3)
    xqp[:, :, :NW] = xq
    xt8 = np.stack([_swz(xt[i]) for i in range(B)])
    xq8 = np.stack([_swz(xqp[i]) for i in range(B)])
    wqkt = _swz(
        (in_proj_w[: 2 * D].T * np.float32(WS8)).astype(ml_dtypes.float8_e4m3)
    )

    # fused rank-2 value path (computed in float64 host-side):
    # wvp[(h,c), :] = (wh @ wo)[c, hslice] @ wv[hslice, :]
    wv = in_proj_w[2 * D :].astype(np.float64)
    bv = in_proj_b[2 * D :].astype(np.float64)
    wo64 = out_proj_w.astype(np.float64)
    wh64 = out_w.astype(np.float64)
    wf = wh64 @ wo64                      # [2, D]
    wvp = np.zeros((2 * H, D), np.float64)
    for h in range(H):
        sl_ = slice(h * HD, (h + 1) * HD)
        wvp[2 * h : 2 * h + 2] = wf[:, sl_] @ wv[sl_, :]
    bc2 = (wh64 @ (wo64 @ bv + out_proj_b.astype(np.float64))
           + out_b.astype(np.float64)).astype(np.float32)
    wvpt = _swz((wvp.T * VS8).astype(ml_dtypes.float8_e4m3))
    bqk8 = _swz(
        (in_proj_b[: 2 * D] * np.float32(WS8)).reshape(2 * D, 1)
    ).reshape(128, 16)

    in_maps = []
    for c in range(NCORES):
        in_maps.append({
            "xt": np.ascontiguousarray(xt8[c * BL : (c + 1) * BL]),
            "xq": np.ascontiguousarray(xq8[c * BL : (c + 1) * BL]),
            "wqkt": wqkt,
            "wvpt": wvpt,
            "bqk": bqk8,
            "bc2": bc2,
        })

    if _debug:
        kr = run_bass_kernel_spmd(nc, in_maps[:1], core_ids=[0])
        return kr.results[0]
    kr = run_bass_kernel_spmd(
        nc, in_maps, core_ids=list(range(NCORES)), trace=_want_perf
    )
    out = np.concatenate([r["out"] for r in kr.results], axis=0).reshape(-1, NC)
    if _want_perf:
        return out, kr
    return out


# revision 23
# speedup vs baseline: 1.4032x; 1.0434x over previous
"""Trainium2 Bass kernel for nn_ExpWindowAttention (windowed sparse attention).

Strategy: pure data-parallel over batch (32 -> 8 cores x 4 batches).

Key algebraic fusion: only probs = (o @ wo.T + bo) @ wh.T + bh is consumed
(nC=2), so the whole post-softmax pipeline collapses to rank-2 per head:
  probs[w, c] = sum_{h,t} attn[w, h, t] * vp[5w + t, h, c] + BC[c]
  vp = x @ wvp.T,  wvp[(h, c), :] = (wh @ wo)[c, hslice] @ wv[hslice, :]
This eliminates the V projection, attention-times-V, out-projection, and
head GEMMs entirely; vp is a 16-wide projection and the banded contraction
runs on the vector engine.

Q/K/vp projections run in fp8 (e4m3) with DoubleRow matmuls. Weights are
pre-scaled into e4m3's normal range; the score scale folds into the
softmax exp scale and the vp scale into the PSUM evacuation copy.

Scores are computed band-compactly: windows are processed in packs of
4x32, where each 32-window group gets its own 32-column strip of the PE
array (tile_position col-tiling) against its own 176-position k-slice.
The four group matmuls execute concurrently in the array, producing a
[128, 176] per-head score tile whose 11-wide diagonal band is extracted
with flat-AP DMA gathers; softmax runs in the compact band domain.

All host-side tensors are pre-swizzled to the SBUF partition-major layout
so every load is 128 contiguous descriptors (fast HWDGE dispatch), and
startup DMAs are spread across the sync/scalar/gpsimd queues.
"""

import numpy as np
import ml_dtypes

import concourse.bass as bass
from concourse import bacc
import concourse.mybir as mybir
import concourse.tile as tile
from concourse.bass_utils import run_bass_kernel_spmd

F32 = mybir.dt.float32
BF16 = mybir.dt.bfloat16
F8 = mybir.dt.float8e4
DR = mybir.MatmulPerfMode.DoubleRow
WS8 = 128.0              # fp8 weight pre-scale for q/k projections
VS8 = 1024.0             # fp8 weight pre-scale for the vp projection

NCORES = 8
B = 32
BL = B // NCORES          # batches per core
L = 2048
D = 1024
H = 8
HD = 128
W = 5
T = 2 * W + 1             # 11
NW = 409                  # windows per batch
NWQ = 416                 # padded center count (32-mult)
NC = 2
LP = 2176                 # padded position count (17*128)
LK = 2056                 # k/vp positions actually needed (5*408+10+1)
SCALE = float(1.0 / np.sqrt(HD))
SSCALE = float(SCALE / (WS8 * WS8))   # scores carry WS8^2 from fp8 q/k scaling
NEGLOG2 = float(-np.log(2.0))

# non-overlapping K/vp projection segments covering [0, LK)
SEGS = [(0, 512), (512, 512), (1024, 512), (1536, 416), (1952, 104)]
# score chunks: (first window, window count, first position, k-width)
CHUNKS = [(0, 96, 0, 512), (96, 96, 480, 512), (192, 96, 960, 512),
          (288, 96, 1440, 512), (384, 25, 1920, 136)]


def _flat_ap(t, extra_offset, dims):
    """AP over a tile's backing tensor flat element space (partition-major)."""
    return bass.AP(tensor=t.tensor, offset=t.offset + extra_offset,
                   ap=[list(d) for d in dims])


def _rowsz(t):
    """True per-partition stride (elements) of a tile, from its own AP."""
    return int(t[:].ap[0][0])


def _build(debug=False):
    nc = bacc.Bacc(None, target_bir_lowering=False)

    xt_d = nc.declare_dram_parameter("xt", [BL, 128, 8 * LP], F8, isOutput=False)
    xq_d = nc.declare_dram_parameter("xq", [BL, 128, 8 * NWQ], F8, isOutput=False)
    wqk_d = nc.declare_dram_parameter("wqkt", [128, 8 * 2 * D], F8, isOutput=False)
    wvp_d = nc.declare_dram_parameter("wvpt", [128, 8 * 2 * H], F8, isOutput=False)
    bqk_d = nc.declare_dram_parameter("bqk", [128, 16], F32, isOutput=False)
    bc2_d = nc.declare_dram_parameter("bc2", [NC], F32, isOutput=False)
    out_d = nc.declare_dram_parameter("out", [BL, L, NC], F32, isOutput=True)

    with tile.TileContext(nc) as tc:
        import contextlib
        with contextlib.ExitStack() as ctx:
            const = ctx.enter_context(tc.tile_pool(name="const", bufs=1))
            perb = ctx.enter_context(tc.tile_pool(name="perb", bufs=2))
            perb2 = ctx.enter_context(tc.tile_pool(name="perb2", bufs=2))
            tailp = ctx.enter_context(tc.tile_pool(name="tailp", bufs=4))
            xtp = ctx.enter_context(tc.tile_pool(name="xtp", bufs=2))
            ktp = ctx.enter_context(tc.tile_pool(name="ktp", bufs=2))
            vpsp = ctx.enter_context(tc.tile_pool(name="vpsp", bufs=2))
            drp = ctx.enter_context(tc.tile_pool(name="drp", bufs=2, space="DRAM"))
            ssbp = ctx.enter_context(tc.tile_pool(name="ssbp", bufs=3))
            smx = ctx.enter_context(tc.tile_pool(name="smx", bufs=2))
            proj_ps = ctx.enter_context(tc.tile_pool(name="proj_ps", bufs=3, space="PSUM"))
            vp_ps = ctx.enter_context(tc.tile_pool(name="vp_ps", bufs=2, space="PSUM"))
            sc_ps = ctx.enter_context(tc.tile_pool(name="sc_ps", bufs=3, space="PSUM"))

            # ---- resident weights / biases (scalar queue; sync queue kept
            # free for the latency-critical per-pack gathers) ----
            xqall = const.tile([128, BL * 8, NWQ], F8)
            nc.scalar.dma_start(out=xqall[:, 0:8, :], in_=xq_d[0])
            wqk = const.tile([128, 8, 2 * D], F8)
            for j in range(4):
                nc.scalar.dma_start(
                    out=wqk[:, 2 * j : 2 * j + 2, :],
                    in_=wqk_d[:, 2 * j * 2 * D : (2 * j + 2) * 2 * D],
                )
            wvp = const.tile([128, 8, 2 * H], F8)
            nc.scalar.dma_start(out=wvp, in_=wvp_d[:, :])
            bqk_col = const.tile([128, 16], F32)  # [p, proj*8+fc]
            nc.scalar.dma_start(out=bqk_col, in_=bqk_d[:, :])
            bias2 = const.tile([128, NC], F32)    # BC broadcast along partitions
            nc.scalar.dma_start(
                out=bias2, in_=bass.AP(tensor=bc2_d, offset=0, ap=[[0, 128], [1, NC]])
            )
            cfill = const.tile([128, 8], F32)
            nc.vector.memset(cfill, NEGLOG2)
            xqall_rest_loaded = [False]

            def load_xq_rest():
                for b in range(1, BL):
                    nc.scalar.dma_start(
                        out=xqall[:, b * 8 : (b + 1) * 8, :], in_=xq_d[b]
                    )

            state = {}

            def qproj(b):
                """Per-batch Q projection from host-pregathered centers."""
                qt = perb.tile([128, 8, NWQ], BF16, tag="qt")
                for h in range(8):
                    qps = proj_ps.tile([128, NWQ], F32, tag="pps")
                    for j in range(4):
                        nc.tensor.matmul(
                            qps[:],
                            wqk[:, 2 * j : 2 * j + 2, h * 128 : h * 128 + 128],
                            xqall[:, b * 8 + 2 * j : b * 8 + 2 * j + 2, :],
                            start=(j == 0),
                            stop=(j == 3),
                            perf_mode=DR,
                        )
                    if h % 2 == 0:
                        nc.vector.tensor_scalar_add(
                            qt[:, h, :], qps[:], bqk_col[:, h : h + 1]
                        )
                    else:
                        nc.scalar.add(qt[:, h, :], qps[:], bqk_col[:, h : h + 1])
                state[("qt", b)] = qt

            def load_x(b):
                """Whole-batch transposed-x load (one large contiguous DMA)."""
                xt = xtp.tile([128, 8, LP], F8, tag="xt")
                nc.sync.dma_start(out=xt, in_=xt_d[b])
                state[("xt", b)] = xt

            def produce(b, si):
                """K projection + vp (rank-2 value path) for one (batch, seg)."""
                ss, sl = SEGS[si]
                xt = state[("xt", b)]
                kt = state[("kt", b)]
                for h in range(8):
                    kps = proj_ps.tile([128, sl], F32, tag="pps")
                    for j in range(4):
                        nc.tensor.matmul(
                            kps[:],
                            wqk[:, 2 * j : 2 * j + 2, D + h * 128 : D + h * 128 + 128],
                            xt[:, 2 * j : 2 * j + 2, ss : ss + sl],
                            start=(j == 0),
                            stop=(j == 3),
                            perf_mode=DR,
                        )
                    if h < 4:
                        nc.vector.tensor_scalar_add(
                            kt[:, h, ss : ss + sl], kps[:], bqk_col[:, 8 + h : 9 + h]
                        )
                    else:
                        nc.scalar.add(
                            kt[:, h, ss : ss + sl], kps[:], bqk_col[:, 8 + h : 9 + h]
                        )
                # vp seg: [16, sl] = wvp.T @ x (fp8 DoubleRow); the VS8 weight
                # pre-scale is undone for free in the PSUM evacuation. Staged
                # via DRAM as [c, pos, h]: SBUF-source flat-AP gathers with
                # sub-row partition steps fail BIR verification, DRAM APs are
                # unrestricted.
                vp_d = state[("vp_d", b)]
                vps = vp_ps.tile([2 * H, sl], F32, tag="vps")
                for j in range(4):
                    nc.tensor.matmul(
                        vps[:],
                        wvp[:, 2 * j : 2 * j + 2, :],
                        xt[:, 2 * j : 2 * j + 2, ss : ss + sl],
                        start=(j == 0),
                        stop=(j == 3),
                        perf_mode=DR,
                    )
                vpsb = vpsp.tile([2 * H, sl], F32, tag="vpsb")
                nc.vector.tensor_scalar_mul(vpsb[:], vps[:], float(1.0 / VS8))
                rsv = _rowsz(vpsb)
                for c in range(NC):
                    nc.sync.dma_start(
                        out=_flat_ap(vp_d, c * 8 * LP + ss, [[LP, 8], [1, sl]]),
                        in_=_flat_ap(vpsb, c * rsv, [[2 * rsv, 8], [1, sl]]),
                    )

            def attend(b, ci):
                """Dense per-head scores for one chunk of windows, single
                flat-AP band gather, softmax, banded value contraction."""
                wst, wcnt, pst, kw = CHUNKS[ci]
                qt = state[("qt", b)]
                kt = state[("kt", b)]

                ssb = ssbp.tile([wcnt, 8, kw], F32, tag="ssb")
                for h in range(8):
                    sps = sc_ps.tile([wcnt, kw], F32, tag="sps")
                    nc.tensor.matmul(
                        sps[:], qt[:, h, wst : wst + wcnt],
                        kt[:, h, pst : pst + kw],
                        start=True, stop=True,
                    )
                    if h < 4:
                        nc.vector.tensor_copy(ssb[:, h, :], sps[:])
                    else:
                        nc.scalar.copy(ssb[:, h, :], sps[:])

                # single flat-AP band gather over all heads
                band = smx.tile([wcnt, 8, T], F32, tag="band")
                rs = _rowsz(ssb)
                nc.sync.dma_start(
                    out=band[:],
                    in_=_flat_ap(ssb, 0, [[rs + 5, wcnt], [kw, 8], [1, T]]),
                )

                # softmax in band domain
                negmax = smx.tile([wcnt, 8], F32, tag="negmax")
                nc.vector.tensor_reduce(
                    negmax[:], band[:], axis=mybir.AxisListType.X,
                    op=mybir.AluOpType.max, negate=True,
                )
                ebs = smx.tile([wcnt, 8, T], F32, tag="ebs")
                negmax_bc = negmax[:].unsqueeze(2).to_broadcast([wcnt, 8, T])
                nc.gpsimd.tensor_tensor(
                    out=ebs[:], in0=band[:], in1=negmax_bc,
                    op=mybir.AluOpType.add,
                )
                eb = smx.tile([wcnt, 8, T], F32, tag="eb")
                nc.scalar.activation(
                    eb[:], ebs[:], mybir.ActivationFunctionType.Exp,
                    bias=0.0, scale=SSCALE,
                )
                sums = smx.tile([wcnt, 8], F32, tag="sums")
                nc.vector.tensor_reduce(
                    sums[:], eb[:], axis=mybir.AxisListType.X,
                    op=mybir.AluOpType.add,
                )
                recip = smx.tile([wcnt, 8], F32, tag="recip")
                nc.vector.reciprocal(recip[:], sums[:])
                attn = smx.tile([wcnt, 8, T], F32, tag="attn")
                recip_bc = recip[:].unsqueeze(2).to_broadcast([wcnt, 8, T])
                nc.gpsimd.tensor_tensor(
                    out=attn[:], in0=eb[:], in1=recip_bc,
                    op=mybir.AluOpType.mult,
                )

                # banded value contraction: probs[w, c] = sum_{h,t} attn * vp
                vp_d = state[("vp_d", b)]
                vpb = smx.tile([wcnt, NC, 8, T], F32, tag="vpb")
                for c in range(NC):
                    nc.sync.dma_start(
                        out=vpb[:, c, :, :],
                        in_=_flat_ap(
                            vp_d, 5 * wst + c * 8 * LP,
                            [[5, wcnt], [LP, 8], [1, T]],
                        ),
                    )
                prod = smx.tile([wcnt, NC, 8, T], F32, tag="prod")
                attn_bc = attn[:].unsqueeze(1).to_broadcast([wcnt, NC, 8, T])
                nc.gpsimd.tensor_tensor(
                    out=prod[:], in0=vpb[:], in1=attn_bc,
                    op=mybir.AluOpType.mult,
                )
                red1 = smx.tile([wcnt, NC, 8], F32, tag="red1")
                nc.vector.tensor_reduce(
                    red1[:], prod[:], axis=mybir.AxisListType.X,
                    op=mybir.AluOpType.add,
                )
                pb = state[("pb", b)]
                nc.vector.tensor_reduce(
                    pb[0:wcnt, ci, :], red1[:],
                    axis=mybir.AxisListType.X, op=mybir.AluOpType.add,
                )

            def tail_a(b):
                """per-batch tail up to the Exp accumulation (Exp table only,
                which is already resident from the softmax)."""
                pb = state.pop(("pb", b))
                state.pop(("qt", b))
                state.pop(("vp_d", b))
                state.pop(("xt", b))
                state.pop(("kt", b))
                pb2 = tailp.tile([96, 5, NC], F32, tag="pb2", name=f"pb2_{b}")
                bias_bc = bias2[0:96, :].unsqueeze(1).to_broadcast([96, 5, NC])
                nc.gpsimd.tensor_tensor(
                    out=pb2[:], in0=pb[:], in1=bias_bc, op=mybir.AluOpType.add
                )
                pmax = tailp.tile([96, 5], F32, tag="pmax", name=f"pmax_{b}")
                nc.vector.tensor_reduce(
                    pmax[:], pb2[:], axis=mybir.AxisListType.X, op=mybir.AluOpType.max
                )
                nmax = perb2.tile([96, 5], F32, tag="nmax")
                nc.gpsimd.tensor_scalar_mul(nmax[:], pmax[:], -1.0)
                sexp = tailp.tile([96, 5], F32, tag="sexp", name=f"sexp_{b}")
                etmp = perb2.tile([96, 5, NC], F32, tag="etmp")
                for p in range(5):
                    nc.scalar.activation(
                        etmp[:, p, :], pb2[:, p, :],
                        mybir.ActivationFunctionType.Exp,
                        bias=nmax[:, p : p + 1], scale=1.0,
                        accum_out=sexp[:, p : p + 1],
                    )
                state[("pb2", b)] = pb2
                state[("pmax", b)] = pmax
                state[("sexp", b)] = sexp

            def tail_b(b):
                """deferred Ln + final subtract + output scatter (the Ln
                activation table loads once here, after all softmax Exps)."""
                pb2 = state.pop(("pb2", b))
                pmax = state.pop(("pmax", b))
                sexp = state.pop(("sexp", b))
                lns = perb2.tile([96, 5], F32, tag="lns")
                nc.scalar.activation(lns[:], sexp[:], mybir.ActivationFunctionType.Ln)
                otm2 = state.pop(("otm2", b))
                for p in range(5):
                    nc.gpsimd.tensor_scalar(
                        out=otm2[:, p, 0:NC],
                        in0=pb2[:, p, :],
                        scalar1=pmax[:, p : p + 1],
                        scalar2=lns[:, p : p + 1],
                        op0=mybir.AluOpType.subtract,
                        op1=mybir.AluOpType.subtract,
                    )

                # 10-wide rows carry [log_softmax, 8x log(1/2) fill]: one
                # contiguous-run scatter covers window + non-window rows
                rt = _rowsz(otm2)
                nc.sync.dma_start(
                    out=bass.AP(
                        tensor=out_d, offset=b * L * NC,
                        ap=[[10, 96], [960, 4], [1, 10]],
                    ),
                    in_=_flat_ap(otm2, 0, [[rt, 96], [10, 4], [1, 10]]),
                )
                nc.sync.dma_start(
                    out=bass.AP(
                        tensor=out_d, offset=b * L * NC + 3840, ap=[[10, 25], [1, 10]]
                    ),
                    in_=_flat_ap(otm2, 4 * 10, [[rt, 25], [1, 10]]),
                )
                nc.sync.dma_start(
                    out=bass.AP(
                        tensor=out_d, offset=b * L * NC + 2045 * NC, ap=[[1, 6]]
                    ),
                    in_=cfill[0:1, 0:6],
                )

            def start_batch(b):
                load_x(b)
                qproj(b)
                state[("kt", b)] = ktp.tile([128, 8, LK], BF16, tag="kt", name="kt")
                state[("vp_d", b)] = drp.tile(
                    [NC, 8, LP], F32, tag="vp_d", name="vp_d"
                )
                state[("pb", b)] = perb.tile([96, 5, NC], F32, tag="pb", name="pb")
                otm2 = tailp.tile([96, 5, 10], F32, tag="otm2", name=f"otm2_{b}")
                nc.vector.memset(otm2, NEGLOG2)
                state[("otm2", b)] = otm2

            # ---- software-pipelined emission ----
            # pack p needs segs {p, p+1} (p<3) / {3, 4} (p=3)
            start_batch(0)
            for b in range(BL):
                produce(b, 0)
                produce(b, 1)
                if b == 0:
                    load_xq_rest()
                attend(b, 0)
                if b + 1 < BL:
                    start_batch(b + 1)
                produce(b, 2)
                attend(b, 1)
                produce(b, 3)
                attend(b, 2)
                produce(b, 4)
                attend(b, 4)
                attend(b, 3)
                tail_a(b)
            for b in range(BL):
                tail_b(b)

    nc.compile()
    return nc


_NC_CACHE = {}


def _get_nc(debug=False):
    if debug not in _NC_CACHE:
        _NC_CACHE[debug] = _build(debug)
    return _NC_CACHE[debug]


def _swz(a):
    """[X*128, C] -> [128, X*C] partition-major pre-swizzle (X=8 groups)."""
    x, c = a.shape[0] // 128, a.shape[1]
    return np.ascontiguousarray(
        a.reshape(x, 128, c).transpose(1, 0, 2).reshape(128, x * c)
    )


def kernel(x, in_proj_w, in_proj_b, out_proj_w, out_proj_b, out_w, out_b, x_len=None,
           _want_perf=False, _debug=False):
    x = np.asarray(x, dtype=np.float32)
    in_proj_w = np.asarray(in_proj_w, dtype=np.float32)
    in_proj_b = np.asarray(in_proj_b, dtype=np.float32)
    out_proj_w = np.asarray(out_proj_w, dtype=np.float32)
    out_proj_b = np.asarray(out_proj_b, dtype=np.float32)
    out_w = np.asarray(out_w, dtype=np.float32)
    out_b = np.asarray(out_b, dtype=np.float32)

    nc = _get_nc(_debug)

    # host-side layout prep; x and q/k weights go to fp8 (e4m3) for
    # DoubleRow matmuls, with weights pre-scaled by WS8 to center the
    # small-magnitude weight distribution in e4m3's normal range. All
    # device tensors are pre-swizzled to SBUF partition-major layout.
    xt = np.zeros((B, D, LP), dtype=ml_dtypes.float8_e4m3)
    xt[:, :, :L] = x.transpose(0, 2, 1).astype(ml_dtypes.float8_e4m3)
    xq = np.ascontiguousarray(xt[:, :, 5 : 5 * NW + 5 : 5])
    xqp = np.zeros((B, D, NWQ), dtype=ml_dtypes.float8_e4m3)
    xqp[:, :, :NW] = xq
    xt8 = np.stack([_swz(xt[i]) for i in range(B)])
    xq8 = np.stack([_swz(xqp[i]) for i in range(B)])
    wqkt = _swz(
        (in_proj_w[: 2 * D].T * np.float32(WS8)).astype(ml_dtypes.float8_e4m3)
    )

    # fused rank-2 value path (computed in float64 host-side):
    # wvp[(h,c), :] = (wh @ wo)[c, hslice] @ wv[hslice, :]
    wv = in_proj_w[2 * D :].astype(np.float64)
    bv = in_proj_b[2 * D :].astype(np.float64)
    wo64 = out_proj_w.astype(np.float64)
    wh64 = out_w.astype(np.float64)
    wf = wh64 @ wo64                      # [2, D]
    wvp = np.zeros((2 * H, D), np.float64)
    for h in range(H):
        sl_ = slice(h * HD, (h + 1) * HD)
        wvp[2 * h : 2 * h + 2] = wf[:, sl_] @ wv[sl_, :]
    bc2 = (wh64 @ (wo64 @ bv + out_proj_b.astype(np.float64))
           + out_b.astype(np.float64)).astype(np.float32)
    wvpt = _swz((wvp.T * VS8).astype(ml_dtypes.float8_e4m3))
    bqk8 = _swz(
        (in_proj_b[: 2 * D] * np.float32(WS8)).reshape(2 * D, 1)
    ).reshape(128, 16)

    in_maps = []
    for c in range(NCORES):
        in_maps.append({
            "xt": np.ascontiguousarray(xt8[c * BL : (c + 1) * BL]),
            "xq": np.ascontiguousarray(xq8[c * BL : (c + 1) * BL]),
            "wqkt": wqkt,
            "wvpt": wvpt,
            "bqk": bqk8,
            "bc2": bc2,
        })

    if _debug:
        kr = run_bass_kernel_spmd(nc, in_maps[:1], core_ids=[0])
        return kr.results[0]
    kr = run_bass_kernel_spmd(
        nc, in_maps, core_ids=list(range(NCORES)), trace=_want_perf
    )
    out = np.concatenate([r["out"] for r in kr.results], axis=0).reshape(-1, NC)
    if _want_perf:
        return out, kr
    return out
